# revision 12
# baseline (speedup 1.0000x reference)
import sys
import os

sys.path.insert(0, "/opt/trn_rl_repo")

from contextlib import ExitStack

import numpy as np
import ml_dtypes

import concourse.bass as bass
import concourse.tile as tile
from concourse import bacc, mybir
from concourse.bass_utils import run_bass_kernel_spmd

BF16 = mybir.dt.float16  # 16-bit compute dtype (fp16: more mantissa than bf16)
FP32 = mybir.dt.float32
FP8 = mybir.dt.float8e4
AF = mybir.ActivationFunctionType
ALU = mybir.AluOpType
npbf16 = np.float16
npfp8 = ml_dtypes.float8_e4m3

ABLATE = int(os.environ.get("ABLATE", "99"))
KDEBUG = int(os.environ.get("KDEBUG", "0"))
NCORES = 8
B = 256          # graphs total
NPG = 128        # nodes per graph
EPG = 1024       # edges per graph
GPC = B // NCORES  # 32 graphs per core
NQ = GPC // 4      # 8 quads per core
F = 32
H = 64
NSTEPS = 2


def _slotcol(g):
    # column order of graphs in uT / XaT / beff tensors (m-major)
    q, m = g // 4, g % 4
    return 8 * m + q


_SLOT = [_slotcol(g) for g in range(GPC)]
_INV_SLOT = [0] * GPC
for _g, _s in enumerate(_SLOT):
    _INV_SLOT[_s] = _g


# ----------------------------------------------------------------------------
# Device graph
# ----------------------------------------------------------------------------

def _build_graph():
    nc = bacc.Bacc(
        "TRN2",
        target_bir_lowering=False,
        debug=False,
        enable_asserts=False,
        num_devices=NCORES,
    )

    def din(name, shape, dt):
        return nc.dram_tensor(name, shape, dt, kind="ExternalInput").ap()

    # per-set big inputs
    xh_d = [din(f"xh{s}", [NQ * 128, 128], BF16) for s in range(2)]
    eh_d = [din(f"eh{s}", [NQ * 2 * 128, 512], BF16) for s in range(2)]
    dsT_d = [din(f"dsT{s}", [GPC * 128, EPG], FP8) for s in range(2)]
    de_d = [din(f"de{s}", [GPC * 128, EPG], FP8) for s in range(2)]
    uT_d = [din(f"uT{s}", [F, GPC], FP32) for s in range(2)]

    # weights (bf16 compute path)
    w1x2_d = din("w1x2", [128, 2 * H], BF16)  # W1 x-part in half-band layout
    dw1e_d = din("dw1e", [128, 128], BF16)   # diag2(W1 e-part), dup'd rows 0-63/64-127
    dw2_d = din("dw2", [128, 128], BF16)     # diag2(W2)
    dw3_d = din("dw3", [128, H], BF16)       # diag2(W3)
    dwnx_d = din("dwnx", [128, 128], BF16)   # diag2(Wn x-part) dup'd
    dwne_d = din("dwne", [128, 128], BF16)   # diag2(Wn eagg-part) dup'd
    dwn2_d = din("dwn2", [128, 128], BF16)
    dwn3_d = din("dwn3", [128, H], BF16)
    dwax_d = din("dwax", [128, 128], BF16)   # diag2(Wa x-part) dup'd
    dwa2_d = din("dwa2", [128, 128], BF16)
    dwa3_d = din("dwa3", [128, H], BF16)
    ident_d = din("ident", [128, 128], BF16)

    # fp32 u-path weights
    w1u_d = din("w1u", [F, H], FP32)
    wnuo_d = din("wnuo", [F, H], FP32)
    wnuu_d = din("wnuu", [F, H], FP32)
    wau_d = din("wau", [F, H], FP32)
    wgx_d = din("wgx", [F, H], FP32)
    wgu_d = din("wgu", [F, H], FP32)
    wg2_d = din("wg2", [H, H], FP32)
    wg3_d = din("wg3", [H, F], FP32)
    wo1_d = din("wo1", [F, H], FP32)
    wo2_d = din("wo2", [F, H], FP32)
    wo_2_d = din("wo_2", [H, H], FP32)
    wo_3_d = din("wo_3", [H, F], FP32)

    # biases
    b1_d = din("b1", [H, 1], FP32)
    b2p_d = din("b2p", [128, 1], FP32)
    b3q_d = din("b3q", [128, 1], FP32)
    bn1_d = din("bn1", [H, 1], FP32)
    bn2p_d = din("bn2p", [128, 1], FP32)
    bn3q_d = din("bn3q", [128, 1], FP32)
    ba1_d = din("ba1", [H, 1], FP32)
    ba2p_d = din("ba2p", [128, 1], FP32)
    ba3q_d = din("ba3q", [128, 1], FP32)
    bg1_d = din("bg1", [H, 1], FP32)
    bg2_d = din("bg2", [H, 1], FP32)
    bg3_d = din("bg3", [F, 1], FP32)
    bo1_d = din("bo1", [H, 1], FP32)
    bo2_d = din("bo2", [H, 1], FP32)
    bo3_d = din("bo3", [F, 1], FP32)

    out_d = nc.dram_tensor("out", [NSTEPS * F, GPC], FP32, kind="ExternalOutput").ap()
    dbg_d = {}
    if KDEBUG:
        for nm, shp in [("pb1", [128, 16]), ("y0", [128, H]), ("rh1", [128, 512]),
                        ("rh2", [128, 512]), ("enew", [128, 512]), ("tsb0", [128, 128]),
                        ("eagg0", [128, 128]), ("xnew0", [128, 128]), ("a0", [128, 128]),
                        ("u0p", [F, GPC])]:
            dbg_d[nm] = nc.dram_tensor("dbg_" + nm, shp, FP32, kind="ExternalOutput").ap()

    with ExitStack() as ctx:
        tc = ctx.enter_context(tile.TileContext(nc))

        const = ctx.enter_context(tc.tile_pool(name="const", bufs=1))
        persist = ctx.enter_context(tc.tile_pool(name="persist", bufs=1))
        pbig = ctx.enter_context(tc.tile_pool(name="pbig", bufs=5, space="PSUM"))
        pagg = ctx.enter_context(tc.tile_pool(name="pagg", bufs=1, space="PSUM"))
        psmall = ctx.enter_context(tc.tile_pool(name="psmall", bufs=2, space="PSUM"))
        rhp = ctx.enter_context(tc.tile_pool(name="rhp", bufs=4))
        tsbp = ctx.enter_context(tc.tile_pool(name="tsbp", bufs=3))
        ysbp = ctx.enter_context(tc.tile_pool(name="ysbp", bufs=6))
        eaggp = ctx.enter_context(tc.tile_pool(name="eaggp", bufs=2))
        smallp = ctx.enter_context(tc.tile_pool(name="smallp", bufs=4))
        upool = ctx.enter_context(tc.tile_pool(name="upool", bufs=4))

        def cload(name, dram, shape, dt):
            t = const.tile(shape, dt, tag=name)
            nc.sync.dma_start(t[:], dram)
            return t

        w1x2 = cload("w1x2", w1x2_d, [128, 2 * H], BF16)
        dw1e = cload("dw1e", dw1e_d, [128, 128], BF16)
        dw2 = cload("dw2", dw2_d, [128, 128], BF16)
        dw3 = cload("dw3", dw3_d, [128, H], BF16)
        dwnx = cload("dwnx", dwnx_d, [128, 128], BF16)
        dwne = cload("dwne", dwne_d, [128, 128], BF16)
        dwn2 = cload("dwn2", dwn2_d, [128, 128], BF16)
        dwn3 = cload("dwn3", dwn3_d, [128, H], BF16)
        dwax = cload("dwax", dwax_d, [128, 128], BF16)
        dwa2 = cload("dwa2", dwa2_d, [128, 128], BF16)
        dwa3 = cload("dwa3", dwa3_d, [128, H], BF16)
        ident = cload("ident", ident_d, [128, 128], BF16)

        w1u = cload("w1u", w1u_d, [F, H], FP32)
        wnuo = cload("wnuo", wnuo_d, [F, H], FP32)
        wnuu = cload("wnuu", wnuu_d, [F, H], FP32)
        wau = cload("wau", wau_d, [F, H], FP32)
        wgx = cload("wgx", wgx_d, [F, H], FP32)
        wgu = cload("wgu", wgu_d, [F, H], FP32)
        wg2 = cload("wg2", wg2_d, [H, H], FP32)
        wg3 = cload("wg3", wg3_d, [H, F], FP32)
        wo1 = cload("wo1", wo1_d, [F, H], FP32)
        wo2 = cload("wo2", wo2_d, [F, H], FP32)
        wo_2 = cload("wo_2", wo_2_d, [H, H], FP32)
        wo_3 = cload("wo_3", wo_3_d, [H, F], FP32)

        b1 = cload("b1", b1_d, [H, 1], FP32)
        b2p = cload("b2p", b2p_d, [128, 1], FP32)
        b3q = cload("b3q", b3q_d, [128, 1], FP32)
        bn1 = cload("bn1", bn1_d, [H, 1], FP32)
        bn2p = cload("bn2p", bn2p_d, [128, 1], FP32)
        bn3q = cload("bn3q", bn3q_d, [128, 1], FP32)
        ba1 = cload("ba1", ba1_d, [H, 1], FP32)
        ba2p = cload("ba2p", ba2p_d, [128, 1], FP32)
        ba3q = cload("ba3q", ba3q_d, [128, 1], FP32)
        bg1 = cload("bg1", bg1_d, [H, 1], FP32)
        bg2 = cload("bg2", bg2_d, [H, 1], FP32)
        bg3 = cload("bg3", bg3_d, [F, 1], FP32)
        bo1 = cload("bo1", bo1_d, [H, 1], FP32)
        bo2 = cload("bo2", bo2_d, [H, 1], FP32)
        bo3 = cload("bo3", bo3_d, [F, 1], FP32)

        # persistent state tiles
        xh = [[persist.tile([128, 128], BF16, tag=f"xh{s}_{q}", name=f"xh{s}_{q}") for q in range(NQ)]
              for s in range(2)]
        eh = [[persist.tile([128, 512], BF16, tag=f"eh{s}_{i}", name=f"eh{s}_{i}") for i in range(NQ * 2)]
              for s in range(2)]
        dsT = [[persist.tile([128, EPG], FP8, tag=f"dsT{s}_{g}", name=f"dsT{s}_{g}") for g in range(GPC)]
               for s in range(2)]
        de = [[persist.tile([128, EPG], FP8, tag=f"de{s}_{g}", name=f"de{s}_{g}") for g in range(GPC)]
              for s in range(2)]
        for s in range(2):
            for q in range(NQ):
                nc.sync.dma_start(xh[s][q][:], xh_d[s][128 * q:128 * (q + 1), :])
            for i in range(NQ * 2):
                nc.sync.dma_start(eh[s][i][:], eh_d[s][128 * i:128 * (i + 1), :])
            for g in range(GPC):
                nc.sync.dma_start(dsT[s][g][:], dsT_d[s][128 * g:128 * (g + 1), :])
                nc.sync.dma_start(de[s][g][:], de_d[s][128 * g:128 * (g + 1), :])

        uT = []
        for s in range(2):
            t = upool.tile([F, GPC], FP32, tag="u")
            nc.sync.dma_start(t[:], uT_d[s])
            uT.append(t)

        MM = nc.tensor.matmul

        def pairbias(beff, tag):
            # beff [64, GPC] (slot order) -> pb [128, 16] (pair order)
            pb = smallp.tile([128, 16], FP32, tag=tag)
            src = beff[:, 0:32].rearrange("f (b r) -> f b r", b=2)
            dst_hi = pb[0:64, :].rearrange("f (a b) -> f b a", a=8)
            nc.vector.tensor_copy(dst_hi, src[:, :, 0:8])
            src2 = beff[:, 0:32].rearrange("f (b r) -> f b r", b=2)
            dst_lo = pb[64:128, :].rearrange("f (a b) -> f b a", a=8)
            nc.vector.tensor_copy(dst_lo, src2[:, :, 8:16])
            return pb

        def beff_mm(wlist, ulist, bias, tag):
            # returns [64, GPC] fp32 = sum_i wlist[i].T @ ulist[i] + bias
            ps = psmall.tile([H, GPC], FP32, tag="ps_small")
            for i, (w, u) in enumerate(zip(wlist, ulist)):
                MM(ps[:], w[:], u[:], start=(i == 0), stop=(i == len(wlist) - 1))
            be = smallp.tile([H, GPC], FP32, tag=tag)
            nc.scalar.activation(be[:], ps[:], AF.Identity, bias=bias[:, 0:1])
            return be

        _dbg_done = [False]

        def dbg(nm, ap):
            if KDEBUG and not _dbg_done[0] and nm in dbg_d:
                t = smallp.tile(list(ap.shape), FP32, tag="dbgt", name="dbgt")
                nc.vector.tensor_copy(t[:], ap)
                nc.sync.dma_start(dbg_d[nm], t[:])

        def gnn_pass(s):
            u_own, u_oth = uT[s], uT[1 - s]
            if ABLATE < 2:
                return
            be1 = beff_mm([w1u], [u_own], b1, "be1")
            pb1 = pairbias(be1, "pb1")
            ben = beff_mm([wnuo, wnuu], [u_oth, u_own], bn1, "ben")
            pbn = pairbias(ben, "pbn")
            bea = beff_mm([wau], [u_own], ba1, "bea")
            pba = pairbias(bea, "pba")
            dbg("pb1", pb1[:])

            xagg = smallp.tile([128, NQ], FP32, tag="xagg")

            if ABLATE < 3:
                return
            for q in range(NQ):
                xq = xh[s][q]
                # y_m = x_g @ W1x  (node-major), per quad member
                ys = []
                for m in range(4):
                    half = 64 * (m // 2)
                    yp = psmall.tile([128, H], FP32, tag="ps_small")
                    MM(yp[:], xq[half:half + 64, :],
                       w1x2[half:half + 64, H * (m % 2):H * (m % 2) + H])
                    yt = ysbp.tile([128, H], BF16, tag="ysb")
                    nc.vector.tensor_copy(yt[:], yp[:])
                    ys.append(yt)
                    if m == 0 and q == 0:
                        dbg("y0", yt[:])

                agg = pagg.tile([128, 128], FP32, tag="agg")

                if ABLATE < 4:
                    continue
                for t in range(2):
                    ehb = eh[s][2 * q + t]
                    cs = slice(512 * t, 512 * (t + 1))
                    h1ab = pbig.tile([128, 512], FP32, tag="hps")
                    MM(h1ab[:], dw1e[0:64, :], ehb[0:64, :],
                       start=True, stop=False, skip_group_check=True)
                    MM(h1ab[0:64, :], ys[0][:], dsT[s][4 * q][:, cs],
                       start=False, stop=False, tile_position=(0, 0),
                       skip_group_check=True)
                    MM(h1ab[64:128, :], ys[1][:], dsT[s][4 * q + 1][:, cs],
                       start=False, stop=True, tile_position=(0, 64),
                       skip_group_check=True)
                    h1cd = pbig.tile([128, 512], FP32, tag="hps")
                    MM(h1cd[:], dw1e[64:128, :], ehb[64:128, :],
                       start=True, stop=False, skip_group_check=True)
                    MM(h1cd[0:64, :], ys[2][:], dsT[s][4 * q + 2][:, cs],
                       start=False, stop=False, tile_position=(0, 0),
                       skip_group_check=True)
                    MM(h1cd[64:128, :], ys[3][:], dsT[s][4 * q + 3][:, cs],
                       start=False, stop=True, tile_position=(0, 64),
                       skip_group_check=True)

                    rh1ab = rhp.tile([128, 512], BF16, tag="rh")
                    nc.scalar.activation(rh1ab[:], h1ab[:], AF.Relu,
                                         bias=pb1[:, 2 * q:2 * q + 1])
                    rh1cd = rhp.tile([128, 512], BF16, tag="rh")
                    nc.vector.tensor_scalar(rh1cd[:], h1cd[:],
                                            pb1[:, 2 * q + 1:2 * q + 2], 0.0,
                                            op0=ALU.add, op1=ALU.max)

                    if ABLATE < 5:
                        continue
                    if q == 0 and t == 0:
                        dbg("rh1", rh1ab[:])
                    h2ab = pbig.tile([128, 512], FP32, tag="hps")
                    MM(h2ab[:], dw2[:], rh1ab[:])
                    h2cd = pbig.tile([128, 512], FP32, tag="hps")
                    MM(h2cd[:], dw2[:], rh1cd[:])

                    rh2ab = rhp.tile([128, 512], BF16, tag="rh")
                    nc.scalar.activation(rh2ab[:], h2ab[:], AF.Relu,
                                         bias=b2p[:, 0:1])
                    rh2cd = rhp.tile([128, 512], BF16, tag="rh")
                    nc.vector.tensor_scalar(rh2cd[:], h2cd[:],
                                            b2p[:, 0:1], 0.0,
                                            op0=ALU.add, op1=ALU.max)

                    if q == 0 and t == 0:
                        dbg("rh2", rh2ab[:])
                    eps = pbig.tile([128, 512], FP32, tag="hps")
                    MM(eps[0:64, :], dw3[:], rh2ab[:],
                       tile_position=(0, 0), skip_group_check=True)
                    MM(eps[64:128, :], dw3[:], rh2cd[:],
                       tile_position=(0, 64), skip_group_check=True)
                    # e_new written back in place into the e-home block
                    nc.scalar.activation(ehb[:], eps[:], AF.Identity,
                                         bias=b3q[:, 0:1])

                    if ABLATE < 6:
                        continue
                    if q == 0 and t == 0:
                        dbg("enew", ehb[:])
                    for cc in range(4):
                        c = 4 * t + cc
                        tsb = tsbp.tile([128, 128], BF16, tag="tsb")
                        nc.sync.dma_start_transpose(
                            tsb[:], ehb[:, 128 * cc:128 * (cc + 1)])
                        if q == 0 and c == 0:
                            dbg("tsb0", tsb[:])
                        for m in range(4):
                            MM(agg[:, 32 * m:32 * (m + 1)],
                               de[s][4 * q + m][:, 128 * c:128 * (c + 1)],
                               tsb[:, 32 * m:32 * (m + 1)],
                               start=(c == 0 and m == 0),
                               stop=(c == 7 and m == 3),
                               skip_group_check=True)

                if ABLATE < 6:
                    continue
                eagg_nm = eaggp.tile([128, 128], BF16, tag="eagg_nm")
                nc.vector.tensor_copy(eagg_nm[:], agg[:])
                eagg = eaggp.tile([128, 128], BF16, tag="eagg")
                nc.sync.dma_start_transpose(eagg[:], eagg_nm[:])
                if q == 0:
                    dbg("eagg0", eagg[:])

                if ABLATE < 7:
                    continue
                # node MLP
                pn_ab = psmall.tile([128, 128], FP32, tag="ps_small")
                MM(pn_ab[:], dwnx[0:64, :], xq[0:64, :], start=True, stop=False)
                MM(pn_ab[:], dwne[0:64, :], eagg[0:64, :], start=False, stop=True)
                pn_cd = psmall.tile([128, 128], FP32, tag="ps_small")
                MM(pn_cd[:], dwnx[64:128, :], xq[64:128, :], start=True, stop=False)
                MM(pn_cd[:], dwne[64:128, :], eagg[64:128, :], start=False, stop=True)
                rn1ab = rhp.tile([128, 128], BF16, tag="rn")
                nc.scalar.activation(rn1ab[:], pn_ab[:], AF.Relu,
                                     bias=pbn[:, 2 * q:2 * q + 1])
                rn1cd = rhp.tile([128, 128], BF16, tag="rn")
                nc.vector.tensor_scalar(rn1cd[:], pn_cd[:],
                                        pbn[:, 2 * q + 1:2 * q + 2], 0.0,
                                        op0=ALU.add, op1=ALU.max)
                pn2ab = psmall.tile([128, 128], FP32, tag="ps_small")
                MM(pn2ab[:], dwn2[:], rn1ab[:])
                pn2cd = psmall.tile([128, 128], FP32, tag="ps_small")
                MM(pn2cd[:], dwn2[:], rn1cd[:])
                rn2ab = rhp.tile([128, 128], BF16, tag="rn")
                nc.scalar.activation(rn2ab[:], pn2ab[:], AF.Relu, bias=bn2p[:, 0:1])
                rn2cd = rhp.tile([128, 128], BF16, tag="rn")
                nc.vector.tensor_scalar(rn2cd[:], pn2cd[:], bn2p[:, 0:1], 0.0,
                                        op0=ALU.add, op1=ALU.max)
                px = psmall.tile([128, 128], FP32, tag="ps_small")
                MM(px[0:64, :], dwn3[:], rn2ab[:],
                   tile_position=(0, 0), skip_group_check=True)
                MM(px[64:128, :], dwn3[:], rn2cd[:],
                   tile_position=(0, 64), skip_group_check=True)
                # x_new in place
                nc.scalar.activation(xq[:], px[:], AF.Identity, bias=bn3q[:, 0:1])

                if q == 0:
                    dbg("xnew0", xq[:])
                # attention MLP
                pa_ab = psmall.tile([128, 128], FP32, tag="ps_small")
                MM(pa_ab[:], dwax[0:64, :], xq[0:64, :])
                pa_cd = psmall.tile([128, 128], FP32, tag="ps_small")
                MM(pa_cd[:], dwax[64:128, :], xq[64:128, :])
                ra1ab = rhp.tile([128, 128], BF16, tag="rn")
                nc.scalar.activation(ra1ab[:], pa_ab[:], AF.Relu,
                                     bias=pba[:, 2 * q:2 * q + 1])
                ra1cd = rhp.tile([128, 128], BF16, tag="rn")
                nc.vector.tensor_scalar(ra1cd[:], pa_cd[:],
                                        pba[:, 2 * q + 1:2 * q + 2], 0.0,
                                        op0=ALU.add, op1=ALU.max)
                pa2ab = psmall.tile([128, 128], FP32, tag="ps_small")
                MM(pa2ab[:], dwa2[:], ra1ab[:])
                pa2cd = psmall.tile([128, 128], FP32, tag="ps_small")
                MM(pa2cd[:], dwa2[:], ra1cd[:])
                ra2ab = rhp.tile([128, 128], BF16, tag="rn")
                nc.scalar.activation(ra2ab[:], pa2ab[:], AF.Relu, bias=ba2p[:, 0:1])
                ra2cd = rhp.tile([128, 128], BF16, tag="rn")
                nc.vector.tensor_scalar(ra2cd[:], pa2cd[:], ba2p[:, 0:1], 0.0,
                                        op0=ALU.add, op1=ALU.max)
                pa3 = psmall.tile([128, 128], FP32, tag="ps_small")
                MM(pa3[0:64, :], dwa3[:], ra2ab[:],
                   tile_position=(0, 0), skip_group_check=True)
                MM(pa3[64:128, :], dwa3[:], ra2cd[:],
                   tile_position=(0, 64), skip_group_check=True)
                ablk = rhp.tile([128, 128], BF16, tag="rn")
                nc.scalar.activation(ablk[:], pa3[:], AF.Sigmoid, bias=ba3q[:, 0:1])
                if q == 0:
                    dbg("a0", ablk[:])
                axb = rhp.tile([128, 128], BF16, tag="rn")
                nc.vector.tensor_mul(axb[:], ablk[:], xq[:])
                nc.vector.tensor_reduce(xagg[:, q:q + 1], axb[:],
                                        mybir.AxisListType.X, ALU.add)

            # global MLP (fp32)
            if ABLATE < 8:
                return
            XaT = smallp.tile([F, GPC], FP32, tag="XaT")
            for m in range(4):
                nc.vector.tensor_copy(XaT[:, 8 * m:8 * (m + 1)],
                                      xagg[32 * m:32 * (m + 1), :])
            g1p = psmall.tile([H, GPC], FP32, tag="ps_small")
            MM(g1p[:], wgx[:], XaT[:], start=True, stop=False)
            MM(g1p[:], wgu[:], u_own[:], start=False, stop=True)
            g1 = smallp.tile([H, GPC], FP32, tag="g1")
            nc.scalar.activation(g1[:], g1p[:], AF.Relu, bias=bg1[:, 0:1])
            g2p = psmall.tile([H, GPC], FP32, tag="ps_small")
            MM(g2p[:], wg2[:], g1[:])
            g2 = smallp.tile([H, GPC], FP32, tag="g1")
            nc.scalar.activation(g2[:], g2p[:], AF.Relu, bias=bg2[:, 0:1])
            g3p = psmall.tile([F, GPC], FP32, tag="ps_small")
            MM(g3p[:], wg3[:], g2[:])
            unew = upool.tile([F, GPC], FP32, tag="u")
            nc.scalar.activation(unew[:], g3p[:], AF.Identity, bias=bg3[:, 0:1])
            uT[s] = unew
            dbg("u0p", unew[:])
            _dbg_done[0] = True

        for step in range(NSTEPS):
            gnn_pass(0)
            gnn_pass(1)
            o1p = psmall.tile([H, GPC], FP32, tag="ps_small")
            MM(o1p[:], wo1[:], uT[0][:], start=True, stop=False)
            MM(o1p[:], wo2[:], uT[1][:], start=False, stop=True)
            o1 = smallp.tile([H, GPC], FP32, tag="g1")
            nc.scalar.activation(o1[:], o1p[:], AF.Relu, bias=bo1[:, 0:1])
            o2p = psmall.tile([H, GPC], FP32, tag="ps_small")
            MM(o2p[:], wo_2[:], o1[:])
            o2 = smallp.tile([H, GPC], FP32, tag="g1")
            nc.scalar.activation(o2[:], o2p[:], AF.Relu, bias=bo2[:, 0:1])
            o3p = psmall.tile([F, GPC], FP32, tag="ps_small")
            MM(o3p[:], wo_3[:], o2[:])
            oT = smallp.tile([F, GPC], FP32, tag="oT")
            nc.scalar.activation(oT[:], o3p[:], AF.Identity, bias=bo3[:, 0:1])
            nc.sync.dma_start(out_d[F * step:F * (step + 1), :], oT[:])

    nc.compile()
    return nc


_NC = None


def _get_nc():
    global _NC
    if _NC is None:
        _NC = _build_graph()
    return _NC


# ----------------------------------------------------------------------------
# Host-side prep
# ----------------------------------------------------------------------------

def _diag2(w):
    k, m = w.shape
    out = np.zeros((2 * k, 2 * m), np.float32)
    out[:k, :m] = w
    out[k:, m:] = w
    return out


def _dup(w):
    # stack two copies along partitions
    return np.concatenate([w, w], axis=0)


def _prep_weights(edge_mlp, node_mlp, att_mlp, glob_mlp, out_mlp):
    ws = {}
    (W1, b1), (W2, b2), (W3, b3) = [(np.asarray(w, np.float32),
                                     np.asarray(b, np.float32)) for w, b in edge_mlp]
    # edge input layout: [xdiff(32) | u_oth-diff(32, zero) | e(32) | u_own(32)]
    w1x2 = np.zeros((128, 2 * H), np.float32)
    w1x2[0:32, 0:H] = W1[0:32]
    w1x2[32:64, H:2 * H] = W1[0:32]
    w1x2[64:96, 0:H] = W1[0:32]
    w1x2[96:128, H:2 * H] = W1[0:32]
    ws["w1x2"] = w1x2.astype(npbf16)
    ws["dw1e"] = _dup(_diag2(W1[64:96])).astype(npbf16)[:, :]
    # _dup(_diag2(.)) gives [128,128]? diag2 of [32,64] -> [64,128]; dup -> [128,128]
    ws["dw2"] = _diag2(W2).astype(npbf16)
    ws["dw3"] = _diag2(W3).astype(npbf16)
    ws["w1u"] = W1[96:128].astype(np.float32)
    ws["b1"] = b1.reshape(H, 1).astype(np.float32)
    ws["b2p"] = np.concatenate([b2, b2]).reshape(128, 1).astype(np.float32)
    ws["b3q"] = np.concatenate([b3] * 4).reshape(128, 1).astype(np.float32)

    (Wn1, bn1), (Wn2, bn2), (Wn3, bn3) = [(np.asarray(w, np.float32),
                                           np.asarray(b, np.float32)) for w, b in node_mlp]
    # node input layout: [x(32) | u_oth(32) | eagg(32) | u_own(32)]
    ws["dwnx"] = _dup(_diag2(Wn1[0:32])).astype(npbf16)
    ws["dwne"] = _dup(_diag2(Wn1[64:96])).astype(npbf16)
    ws["dwn2"] = _diag2(Wn2).astype(npbf16)
    ws["dwn3"] = _diag2(Wn3).astype(npbf16)
    ws["wnuo"] = Wn1[32:64].astype(np.float32)
    ws["wnuu"] = Wn1[96:128].astype(np.float32)
    ws["bn1"] = bn1.reshape(H, 1).astype(np.float32)
    ws["bn2p"] = np.concatenate([bn2, bn2]).reshape(128, 1).astype(np.float32)
    ws["bn3q"] = np.concatenate([bn3] * 4).reshape(128, 1).astype(np.float32)

    (Wa1, ba1), (Wa2, ba2), (Wa3, ba3) = [(np.asarray(w, np.float32),
                                           np.asarray(b, np.float32)) for w, b in att_mlp]
    ws["dwax"] = _dup(_diag2(Wa1[0:32])).astype(npbf16)
    ws["dwa2"] = _diag2(Wa2).astype(npbf16)
    ws["dwa3"] = _diag2(Wa3).astype(npbf16)
    ws["wau"] = Wa1[32:64].astype(np.float32)
    ws["ba1"] = ba1.reshape(H, 1).astype(np.float32)
    ws["ba2p"] = np.concatenate([ba2, ba2]).reshape(128, 1).astype(np.float32)
    ws["ba3q"] = np.concatenate([ba3] * 4).reshape(128, 1).astype(np.float32)

    (Wg1, bg1), (Wg2, bg2), (Wg3, bg3) = [(np.asarray(w, np.float32),
                                           np.asarray(b, np.float32)) for w, b in glob_mlp]
    ws["wgx"] = Wg1[0:32].astype(np.float32)
    ws["wgu"] = Wg1[32:64].astype(np.float32)
    ws["wg2"] = Wg2.astype(np.float32)
    ws["wg3"] = Wg3.astype(np.float32)
    ws["bg1"] = bg1.reshape(H, 1).astype(np.float32)
    ws["bg2"] = bg2.reshape(H, 1).astype(np.float32)
    ws["bg3"] = bg3.reshape(F, 1).astype(np.float32)

    (Wo1, bo1), (Wo2, bo2), (Wo3, bo3) = [(np.asarray(w, np.float32),
                                           np.asarray(b, np.float32)) for w, b in out_mlp]
    ws["wo1"] = Wo1[0:32].astype(np.float32)
    ws["wo2"] = Wo1[32:64].astype(np.float32)
    ws["wo_2"] = Wo2.astype(np.float32)
    ws["wo_3"] = Wo3.astype(np.float32)
    ws["bo1"] = bo1.reshape(H, 1).astype(np.float32)
    ws["bo2"] = bo2.reshape(H, 1).astype(np.float32)
    ws["bo3"] = bo3.reshape(F, 1).astype(np.float32)

    ws["ident"] = np.eye(128, dtype=npbf16)
    return ws


def _prep_set(x, ei, e, u, core):
    g0 = GPC * core
    nsl = slice(NPG * g0, NPG * (g0 + GPC))
    esl = slice(EPG * g0, EPG * (g0 + GPC))
    xc = np.asarray(x[nsl], np.float32)        # [4096, 32]
    ec = np.asarray(e[esl], np.float32)        # [32768, 32]
    src = np.asarray(ei[0][esl], np.int64)
    dst = np.asarray(ei[1][esl], np.int64)

    xh = np.zeros((NQ * 128, 128), np.float32)
    ehm = np.zeros((NQ * 2 * 128, 512), np.float32)
    dsT = np.zeros((GPC * 128, EPG), np.float32)
    dem = np.zeros((GPC * 128, EPG), np.float32)
    for g in range(GPC):
        q, m = g // 4, g % 4
        xg = xc[NPG * g:NPG * (g + 1)]         # [128, 32]
        xh[128 * q + 32 * m:128 * q + 32 * (m + 1), :] = xg.T
        eg = ec[EPG * g:EPG * (g + 1)]         # [1024, 32]
        for t in range(2):
            i = 2 * q + t
            ehm[128 * i + 32 * m:128 * i + 32 * (m + 1), :] = \
                eg[512 * t:512 * (t + 1)].T
        sg = (src[EPG * g:EPG * (g + 1)] - NPG * (g0 + g)).astype(np.int64)
        dg = (dst[EPG * g:EPG * (g + 1)] - NPG * (g0 + g)).astype(np.int64)
        blk = dsT[128 * g:128 * (g + 1), :]
        np.add.at(blk, (dg, np.arange(EPG)), 1.0)
        np.add.at(blk, (sg, np.arange(EPG)), -1.0)
        dblk = dem[128 * g:128 * (g + 1), :]
        # D edge-major: per 128-edge chunk, rows = local edge, cols = dst node
        for c in range(8):
            sub = np.zeros((128, 128), np.float32)
            sub[np.arange(128), dg[128 * c:128 * (c + 1)]] = 1.0
            dblk[:, 128 * c:128 * (c + 1)] = sub

    uc = np.asarray(u[g0:g0 + GPC], np.float32)  # [32, 32]
    uT = np.zeros((F, GPC), np.float32)
    for g in range(GPC):
        uT[:, _SLOT[g]] = uc[g]
    return (xh.astype(npbf16), ehm.astype(npbf16),
            dsT.astype(npfp8), dem.astype(npfp8), uT)


def _build_in_maps(inputs):
    ws = _prep_weights(inputs["edge_mlp"], inputs["node_mlp"], inputs["att_mlp"],
                       inputs["glob_mlp"], inputs["out_mlp"])
    in_maps = []
    for core in range(NCORES):
        m = dict(ws)
        for s, (xk, eik, ek, uk) in enumerate(
                [("x1", "edge_index1", "e1", "u1"),
                 ("x2", "edge_index2", "e2", "u2")]):
            xh, ehm, dsT, dem, uT = _prep_set(
                np.asarray(inputs[xk]), np.asarray(inputs[eik]),
                np.asarray(inputs[ek]), np.asarray(inputs[uk]), core)
            m[f"xh{s}"] = xh
            m[f"eh{s}"] = ehm
            m[f"dsT{s}"] = dsT
            m[f"de{s}"] = dem
            m[f"uT{s}"] = uT
        in_maps.append(m)
    return in_maps


def _execute(inputs, trace=False):
    nc = _get_nc()
    in_maps = _build_in_maps(inputs)
    res = run_bass_kernel_spmd(nc, in_maps, core_ids=list(range(NCORES)),
                               trace=trace)
    full = np.zeros((NSTEPS, B, F), np.float32)
    for core in range(NCORES):
        o = np.asarray(res.results[core]["out"], np.float32).reshape(NSTEPS, F, GPC)
        for g in range(GPC):
            full[:, GPC * core + g, :] = o[:, :, _SLOT[g]]
    return full, res


def kernel(**inputs):
    out, _ = _execute(inputs, trace=False)
    return out


# revision 22
# speedup vs baseline: 1.2680x; 1.2680x over previous
import sys
import os

sys.path.insert(0, "/opt/trn_rl_repo")

from contextlib import ExitStack

import numpy as np
import ml_dtypes

import concourse.bass as bass
import concourse.tile as tile
from concourse import bacc, mybir
from concourse.bass_utils import run_bass_kernel_spmd

BF16 = mybir.dt.float16  # 16-bit compute dtype (fp16: more mantissa than bf16)
FP32 = mybir.dt.float32
FP8 = mybir.dt.float8e4
AF = mybir.ActivationFunctionType
ALU = mybir.AluOpType
npbf16 = np.float16
npfp8 = ml_dtypes.float8_e4m3

ABLATE = int(os.environ.get("ABLATE", "99"))
KDEBUG = int(os.environ.get("KDEBUG", "0"))
NCORES = 8
B = 256          # graphs total
NPG = 128        # nodes per graph
EPG = 1024       # edges per graph
GPC = B // NCORES  # 32 graphs per core
NQ = GPC // 4      # 8 quads per core
F = 32
H = 64
NSTEPS = 2


def _slotcol(g):
    # column order of graphs in uT / XaT / beff tensors (m-major)
    q, m = g // 4, g % 4
    return 8 * m + q


_SLOT = [_slotcol(g) for g in range(GPC)]
_INV_SLOT = [0] * GPC
for _g, _s in enumerate(_SLOT):
    _INV_SLOT[_s] = _g


# ----------------------------------------------------------------------------
# Device graph
# ----------------------------------------------------------------------------

def _build_graph():
    nc = bacc.Bacc(
        "TRN2",
        target_bir_lowering=False,
        debug=False,
        enable_asserts=False,
        num_devices=NCORES,
    )

    def din(name, shape, dt):
        return nc.dram_tensor(name, shape, dt, kind="ExternalInput").ap()

    # per-set big inputs
    xh_d = [din(f"xh{s}", [NQ * 128, 128], BF16) for s in range(2)]
    eh_d = [din(f"eh{s}", [NQ * 2 * 128, 512], BF16) for s in range(2)]
    dsT_d = [din(f"dsT{s}", [GPC * 128, EPG], FP8) for s in range(2)]
    de_d = [din(f"de{s}", [GPC * 128, EPG], FP8) for s in range(2)]
    uT_d = [din(f"uT{s}", [F, GPC], FP32) for s in range(2)]

    # weights (bf16 compute path)
    w1x2_d = din("w1x2", [128, 2 * H], BF16)  # W1 x-part in half-band layout
    dw1e_d = din("dw1e", [128, 128], BF16)   # diag2(W1 e-part), dup'd rows 0-63/64-127
    dw2_d = din("dw2", [128, 128], BF16)     # diag2(W2)
    dw3_d = din("dw3", [128, H], BF16)       # diag2(W3)
    dwnx_d = din("dwnx", [128, 128], BF16)   # diag2(Wn x-part) dup'd
    dwne_d = din("dwne", [128, 128], BF16)   # diag2(Wn eagg-part) dup'd
    dwn2_d = din("dwn2", [128, 128], BF16)
    dwn3_d = din("dwn3", [128, H], BF16)
    dwax_d = din("dwax", [128, 128], BF16)   # diag2(Wa x-part) dup'd
    dwa2_d = din("dwa2", [128, 128], BF16)
    dwa3_d = din("dwa3", [128, H], BF16)
    w3s2_d = din("w3s2", [128, F], BF16)     # plain W3 dup'd at rows 0-63/64-127
    aggcorr_d = [din(f"aggcorr{s}", [NQ * 128, 128], FP32) for s in range(2)]

    # fp32 u-path weights
    w1u_d = din("w1u", [F, H], FP32)
    wnuo_d = din("wnuo", [F, H], FP32)
    wnuu_d = din("wnuu", [F, H], FP32)
    wau_d = din("wau", [F, H], FP32)
    wgx_d = din("wgx", [F, H], FP32)
    wgu_d = din("wgu", [F, H], FP32)
    wg2_d = din("wg2", [H, H], FP32)
    wg3_d = din("wg3", [H, F], FP32)
    wo1_d = din("wo1", [F, H], FP32)
    wo2_d = din("wo2", [F, H], FP32)
    wo_2_d = din("wo_2", [H, H], FP32)
    wo_3_d = din("wo_3", [H, F], FP32)

    # biases
    b1_d = din("b1", [H, 1], FP32)
    b2p_d = din("b2p", [128, 1], FP32)
    b3q_d = din("b3q", [128, 1], FP32)
    bn1_d = din("bn1", [H, 1], FP32)
    bn2p_d = din("bn2p", [128, 1], FP32)
    bn3q_d = din("bn3q", [128, 1], FP32)
    ba1_d = din("ba1", [H, 1], FP32)
    ba2p_d = din("ba2p", [128, 1], FP32)
    ba3q_d = din("ba3q", [128, 1], FP32)
    bg1_d = din("bg1", [H, 1], FP32)
    bg2_d = din("bg2", [H, 1], FP32)
    bg3_d = din("bg3", [F, 1], FP32)
    bo1_d = din("bo1", [H, 1], FP32)
    bo2_d = din("bo2", [H, 1], FP32)
    bo3_d = din("bo3", [F, 1], FP32)

    out_d = nc.dram_tensor("out", [NSTEPS * F, GPC], FP32, kind="ExternalOutput").ap()
    dbg_d = {}
    if KDEBUG:
        for nm, shp in [("pb1", [128, 16]), ("y0", [128, H]), ("rh1", [128, 512]),
                        ("rh2", [128, 512]), ("enew", [128, 512]), ("tsb0", [128, 128]),
                        ("eagg0", [128, 128]), ("xnew0", [128, 128]), ("a0", [128, 128]),
                        ("u0p", [F, GPC])]:
            dbg_d[nm] = nc.dram_tensor("dbg_" + nm, shp, FP32, kind="ExternalOutput").ap()

    with ExitStack() as ctx:
        tc = ctx.enter_context(tile.TileContext(nc))

        const = ctx.enter_context(tc.tile_pool(name="const", bufs=1))
        persist = ctx.enter_context(tc.tile_pool(name="persist", bufs=1))
        pbig = ctx.enter_context(tc.tile_pool(name="pbig", bufs=5, space="PSUM"))
        pagg = ctx.enter_context(tc.tile_pool(name="pagg", bufs=1, space="PSUM"))
        psmall = ctx.enter_context(tc.tile_pool(name="psmall", bufs=2, space="PSUM"))
        rhp = ctx.enter_context(tc.tile_pool(name="rhp", bufs=4))
        tsbp = ctx.enter_context(tc.tile_pool(name="tsbp", bufs=3))
        ysbp = ctx.enter_context(tc.tile_pool(name="ysbp", bufs=6))
        eaggp = ctx.enter_context(tc.tile_pool(name="eaggp", bufs=2))
        smallp = ctx.enter_context(tc.tile_pool(name="smallp", bufs=4))
        upool = ctx.enter_context(tc.tile_pool(name="upool", bufs=4))

        def cload(name, dram, shape, dt):
            t = const.tile(shape, dt, tag=name)
            nc.sync.dma_start(t[:], dram)
            return t

        w1x2 = cload("w1x2", w1x2_d, [128, 2 * H], BF16)
        dw1e = cload("dw1e", dw1e_d, [128, 128], BF16)
        dw2 = cload("dw2", dw2_d, [128, 128], BF16)
        dw3 = cload("dw3", dw3_d, [128, H], BF16)
        dwnx = cload("dwnx", dwnx_d, [128, 128], BF16)
        dwne = cload("dwne", dwne_d, [128, 128], BF16)
        dwn2 = cload("dwn2", dwn2_d, [128, 128], BF16)
        dwn3 = cload("dwn3", dwn3_d, [128, H], BF16)
        dwax = cload("dwax", dwax_d, [128, 128], BF16)
        dwa2 = cload("dwa2", dwa2_d, [128, 128], BF16)
        dwa3 = cload("dwa3", dwa3_d, [128, H], BF16)
        w3s2 = cload("w3s2", w3s2_d, [128, F], BF16)

        w1u = cload("w1u", w1u_d, [F, H], FP32)
        wnuo = cload("wnuo", wnuo_d, [F, H], FP32)
        wnuu = cload("wnuu", wnuu_d, [F, H], FP32)
        wau = cload("wau", wau_d, [F, H], FP32)
        wgx = cload("wgx", wgx_d, [F, H], FP32)
        wgu = cload("wgu", wgu_d, [F, H], FP32)
        wg2 = cload("wg2", wg2_d, [H, H], FP32)
        wg3 = cload("wg3", wg3_d, [H, F], FP32)
        wo1 = cload("wo1", wo1_d, [F, H], FP32)
        wo2 = cload("wo2", wo2_d, [F, H], FP32)
        wo_2 = cload("wo_2", wo_2_d, [H, H], FP32)
        wo_3 = cload("wo_3", wo_3_d, [H, F], FP32)

        b1 = cload("b1", b1_d, [H, 1], FP32)
        b2p = cload("b2p", b2p_d, [128, 1], FP32)
        b3q = cload("b3q", b3q_d, [128, 1], FP32)
        bn1 = cload("bn1", bn1_d, [H, 1], FP32)
        bn2p = cload("bn2p", bn2p_d, [128, 1], FP32)
        bn3q = cload("bn3q", bn3q_d, [128, 1], FP32)
        ba1 = cload("ba1", ba1_d, [H, 1], FP32)
        ba2p = cload("ba2p", ba2p_d, [128, 1], FP32)
        ba3q = cload("ba3q", ba3q_d, [128, 1], FP32)
        bg1 = cload("bg1", bg1_d, [H, 1], FP32)
        bg2 = cload("bg2", bg2_d, [H, 1], FP32)
        bg3 = cload("bg3", bg3_d, [F, 1], FP32)
        bo1 = cload("bo1", bo1_d, [H, 1], FP32)
        bo2 = cload("bo2", bo2_d, [H, 1], FP32)
        bo3 = cload("bo3", bo3_d, [F, 1], FP32)

        # persistent state tiles
        xh = [[persist.tile([128, 128], BF16, tag=f"xh{s}_{q}", name=f"xh{s}_{q}") for q in range(NQ)]
              for s in range(2)]
        eh = [[persist.tile([128, 512], BF16, tag=f"eh{s}_{i}", name=f"eh{s}_{i}") for i in range(NQ * 2)]
              for s in range(2)]
        dsT = [[persist.tile([128, EPG], FP8, tag=f"dsT{s}_{g}", name=f"dsT{s}_{g}") for g in range(GPC)]
               for s in range(2)]
        de = [[persist.tile([128, EPG], FP8, tag=f"de{s}_{g}", name=f"de{s}_{g}") for g in range(GPC)]
              for s in range(2)]
        acorr = [[persist.tile([128, 128], FP32, tag=f"ac{s}_{q}", name=f"ac{s}_{q}")
                  for q in range(NQ)] for s in range(2)]
        def ldma(dst, srcap):
            nc.sync.dma_start(dst, srcap)

        for s in range(2):
            for q in range(NQ):
                ldma(acorr[s][q][:], aggcorr_d[s][128 * q:128 * (q + 1), :])
            for q in range(NQ):
                ldma(xh[s][q][:], xh_d[s][128 * q:128 * (q + 1), :])
            for i in range(NQ * 2):
                ldma(eh[s][i][:], eh_d[s][128 * i:128 * (i + 1), :])
            for g in range(GPC):
                ldma(dsT[s][g][:], dsT_d[s][128 * g:128 * (g + 1), :])
                ldma(de[s][g][:], de_d[s][128 * g:128 * (g + 1), :])

        uT = []
        for s in range(2):
            t = upool.tile([F, GPC], FP32, tag="u")
            nc.sync.dma_start(t[:], uT_d[s])
            uT.append(t)

        MM = nc.tensor.matmul

        def pairbias(beff, tag):
            # beff [64, GPC] (slot order) -> pb [128, 16] (pair order)
            pb = smallp.tile([128, 16], FP32, tag=tag)
            src = beff[:, 0:32].rearrange("f (b r) -> f b r", b=2)
            dst_hi = pb[0:64, :].rearrange("f (a b) -> f b a", a=8)
            nc.vector.tensor_copy(dst_hi, src[:, :, 0:8])
            src2 = beff[:, 0:32].rearrange("f (b r) -> f b r", b=2)
            dst_lo = pb[64:128, :].rearrange("f (a b) -> f b a", a=8)
            nc.vector.tensor_copy(dst_lo, src2[:, :, 8:16])
            return pb

        def beff_mm(wlist, ulist, bias, tag):
            # returns [64, GPC] fp32 = sum_i wlist[i].T @ ulist[i] + bias
            ps = psmall.tile([H, GPC], FP32, tag="ps_small")
            for i, (w, u) in enumerate(zip(wlist, ulist)):
                MM(ps[:], w[:], u[:], start=(i == 0), stop=(i == len(wlist) - 1))
            be = smallp.tile([H, GPC], FP32, tag=tag)
            nc.scalar.activation(be[:], ps[:], AF.Identity, bias=bias[:, 0:1])
            return be

        _dbg_done = [False]

        def dbg(nm, ap):
            if KDEBUG and not _dbg_done[0] and nm in dbg_d:
                t = smallp.tile(list(ap.shape), FP32, tag="dbgt", name="dbgt")
                nc.vector.tensor_copy(t[:], ap)
                nc.sync.dma_start(dbg_d[nm], t[:])

        def gnn_pass(s):
            u_own, u_oth = uT[s], uT[1 - s]
            if ABLATE < 2:
                return
            be1 = beff_mm([w1u], [u_own], b1, "be1")
            pb1 = pairbias(be1, "pb1")
            ben = beff_mm([wnuo, wnuu], [u_oth, u_own], bn1, "ben")
            pbn = pairbias(ben, "pbn")
            bea = beff_mm([wau], [u_own], ba1, "bea")
            pba = pairbias(bea, "pba")
            dbg("pb1", pb1[:])

            xagg = smallp.tile([128, NQ], FP32, tag="xagg")

            if ABLATE < 3:
                return
            for q in range(NQ):
                xq = xh[s][q]
                # y_m = x_g @ W1x  (node-major), per quad member
                ys = []
                for m in range(4):
                    half = 64 * (m // 2)
                    yp = psmall.tile([128, H], FP32, tag="ps_small")
                    MM(yp[:], xq[half:half + 64, :],
                       w1x2[half:half + 64, H * (m % 2):H * (m % 2) + H])
                    yt = ysbp.tile([128, H], BF16, tag="ysb")
                    nc.vector.tensor_copy(yt[:], yp[:])
                    ys.append(yt)
                    if m == 0 and q == 0:
                        dbg("y0", yt[:])

                agg = None
                if not os.environ.get("NOSCAT"):
                    agg = pagg.tile([128, 128], FP32, tag="agg")

                if ABLATE < 4:
                    continue
                for t in range(2):
                    ehb = eh[s][2 * q + t]
                    cs = slice(512 * t, 512 * (t + 1))
                    h1ab = pbig.tile([128, 512], FP32, tag="hps")
                    MM(h1ab[:], dw1e[0:64, :], ehb[0:64, :],
                       start=True, stop=False, skip_group_check=True)
                    MM(h1ab[0:64, :], ys[0][:], dsT[s][4 * q][:, cs],
                       start=False, stop=False, tile_position=(0, 0),
                       skip_group_check=True)
                    MM(h1ab[64:128, :], ys[1][:], dsT[s][4 * q + 1][:, cs],
                       start=False, stop=True, tile_position=(0, 64),
                       skip_group_check=True)
                    h1cd = pbig.tile([128, 512], FP32, tag="hps")
                    MM(h1cd[:], dw1e[64:128, :], ehb[64:128, :],
                       start=True, stop=False, skip_group_check=True)
                    MM(h1cd[0:64, :], ys[2][:], dsT[s][4 * q + 2][:, cs],
                       start=False, stop=False, tile_position=(0, 0),
                       skip_group_check=True)
                    MM(h1cd[64:128, :], ys[3][:], dsT[s][4 * q + 3][:, cs],
                       start=False, stop=True, tile_position=(0, 64),
                       skip_group_check=True)

                    rh1ab = rhp.tile([128, 512], BF16, tag="rh")
                    nc.scalar.activation(rh1ab[:], h1ab[:], AF.Relu,
                                         bias=pb1[:, 2 * q:2 * q + 1])
                    rh1cd = rhp.tile([128, 512], BF16, tag="rh")
                    nc.vector.tensor_scalar(rh1cd[:], h1cd[:],
                                            pb1[:, 2 * q + 1:2 * q + 2], 0.0,
                                            op0=ALU.add, op1=ALU.max)

                    if ABLATE < 5:
                        continue
                    if q == 0 and t == 0:
                        dbg("rh1", rh1ab[:])
                    h2ab = pbig.tile([128, 512], FP32, tag="hps")
                    MM(h2ab[:], dw2[:], rh1ab[:])
                    h2cd = pbig.tile([128, 512], FP32, tag="hps")
                    MM(h2cd[:], dw2[:], rh1cd[:])

                    rh2ab = rhp.tile([128, 512], BF16, tag="rh")
                    nc.scalar.activation(rh2ab[:], h2ab[:], AF.Relu,
                                         bias=b2p[:, 0:1])
                    rh2cd = rhp.tile([128, 512], BF16, tag="rh")
                    nc.vector.tensor_scalar(rh2cd[:], h2cd[:],
                                            b2p[:, 0:1], 0.0,
                                            op0=ALU.add, op1=ALU.max)

                    if q == 0 and t == 0:
                        dbg("rh2", rh2ab[:])
                    eps = pbig.tile([128, 512], FP32, tag="hps")
                    MM(eps[0:64, :], dw3[:], rh2ab[:],
                       tile_position=(0, 0), skip_group_check=True)
                    MM(eps[64:128, :], dw3[:], rh2cd[:],
                       tile_position=(0, 64), skip_group_check=True)
                    # e_new written back in place into the e-home block
                    nc.scalar.activation(ehb[:], eps[:], AF.Identity,
                                         bias=b3q[:, 0:1])

                    if ABLATE < 6:
                        continue
                    if q == 0 and t == 0:
                        dbg("enew", ehb[:])
                    for cc in range(4):
                        c = 4 * t + cc
                        # e_new edge-major (no b3; corrected at agg evict):
                        # lhsT = pair-stacked rh2 chunk (K=128), rhs = diag2(W3)
                        epsE = pbig.tile([128, 128], FP32, tag="hps")
                        MM(epsE[:, 0:64],
                           rh2ab[:, 128 * cc:128 * (cc + 1)], dw3[:],
                           start=True, stop=False, skip_group_check=True)
                        MM(epsE[:, 64:128],
                           rh2cd[:, 128 * cc:128 * (cc + 1)], dw3[:],
                           start=False, stop=True, skip_group_check=True)
                        if os.environ.get("NOTSB"):
                            continue
                        tsb = tsbp.tile([128, 128], BF16, tag="tsb")
                        nc.vector.tensor_copy(tsb[:], epsE[:])
                        if q == 0 and c == 0:
                            dbg("tsb0", tsb[:])
                        if os.environ.get("NOSCAT"):
                            continue
                        for m in range(4):
                            MM(agg[:, 32 * m:32 * (m + 1)],
                               de[s][4 * q + m][:, 128 * c:128 * (c + 1)],
                               tsb[:, 32 * m:32 * (m + 1)],
                               start=(c == 0 and m == 0),
                               stop=(c == 7 and m == 3),
                               skip_group_check=True)

                if ABLATE < 6 or os.environ.get("NOSCAT"):
                    continue
                eagg_nm = eaggp.tile([128, 128], BF16, tag="eagg_nm")
                nc.vector.tensor_add(eagg_nm[:], agg[:], acorr[s][q][:])
                eagg = eaggp.tile([128, 128], BF16, tag="eagg")
                nc.sync.dma_start_transpose(eagg[:], eagg_nm[:])
                if q == 0:
                    dbg("eagg0", eagg[:])

                if ABLATE < 7:
                    continue
                # node MLP
                pn_ab = psmall.tile([128, 128], FP32, tag="ps_small")
                MM(pn_ab[:], dwnx[0:64, :], xq[0:64, :], start=True, stop=False)
                MM(pn_ab[:], dwne[0:64, :], eagg[0:64, :], start=False, stop=True)
                pn_cd = psmall.tile([128, 128], FP32, tag="ps_small")
                MM(pn_cd[:], dwnx[64:128, :], xq[64:128, :], start=True, stop=False)
                MM(pn_cd[:], dwne[64:128, :], eagg[64:128, :], start=False, stop=True)
                rn1ab = rhp.tile([128, 128], BF16, tag="rn")
                nc.scalar.activation(rn1ab[:], pn_ab[:], AF.Relu,
                                     bias=pbn[:, 2 * q:2 * q + 1])
                rn1cd = rhp.tile([128, 128], BF16, tag="rn")
                nc.vector.tensor_scalar(rn1cd[:], pn_cd[:],
                                        pbn[:, 2 * q + 1:2 * q + 2], 0.0,
                                        op0=ALU.add, op1=ALU.max)
                pn2ab = psmall.tile([128, 128], FP32, tag="ps_small")
                MM(pn2ab[:], dwn2[:], rn1ab[:])
                pn2cd = psmall.tile([128, 128], FP32, tag="ps_small")
                MM(pn2cd[:], dwn2[:], rn1cd[:])
                rn2ab = rhp.tile([128, 128], BF16, tag="rn")
                nc.scalar.activation(rn2ab[:], pn2ab[:], AF.Relu, bias=bn2p[:, 0:1])
                rn2cd = rhp.tile([128, 128], BF16, tag="rn")
                nc.vector.tensor_scalar(rn2cd[:], pn2cd[:], bn2p[:, 0:1], 0.0,
                                        op0=ALU.add, op1=ALU.max)
                px = psmall.tile([128, 128], FP32, tag="ps_small")
                MM(px[0:64, :], dwn3[:], rn2ab[:],
                   tile_position=(0, 0), skip_group_check=True)
                MM(px[64:128, :], dwn3[:], rn2cd[:],
                   tile_position=(0, 64), skip_group_check=True)
                # x_new in place
                nc.scalar.activation(xq[:], px[:], AF.Identity, bias=bn3q[:, 0:1])

                if q == 0:
                    dbg("xnew0", xq[:])
                # attention MLP
                pa_ab = psmall.tile([128, 128], FP32, tag="ps_small")
                MM(pa_ab[:], dwax[0:64, :], xq[0:64, :])
                pa_cd = psmall.tile([128, 128], FP32, tag="ps_small")
                MM(pa_cd[:], dwax[64:128, :], xq[64:128, :])
                ra1ab = rhp.tile([128, 128], BF16, tag="rn")
                nc.scalar.activation(ra1ab[:], pa_ab[:], AF.Relu,
                                     bias=pba[:, 2 * q:2 * q + 1])
                ra1cd = rhp.tile([128, 128], BF16, tag="rn")
                nc.vector.tensor_scalar(ra1cd[:], pa_cd[:],
                                        pba[:, 2 * q + 1:2 * q + 2], 0.0,
                                        op0=ALU.add, op1=ALU.max)
                pa2ab = psmall.tile([128, 128], FP32, tag="ps_small")
                MM(pa2ab[:], dwa2[:], ra1ab[:])
                pa2cd = psmall.tile([128, 128], FP32, tag="ps_small")
                MM(pa2cd[:], dwa2[:], ra1cd[:])
                ra2ab = rhp.tile([128, 128], BF16, tag="rn")
                nc.scalar.activation(ra2ab[:], pa2ab[:], AF.Relu, bias=ba2p[:, 0:1])
                ra2cd = rhp.tile([128, 128], BF16, tag="rn")
                nc.vector.tensor_scalar(ra2cd[:], pa2cd[:], ba2p[:, 0:1], 0.0,
                                        op0=ALU.add, op1=ALU.max)
                pa3 = psmall.tile([128, 128], FP32, tag="ps_small")
                MM(pa3[0:64, :], dwa3[:], ra2ab[:],
                   tile_position=(0, 0), skip_group_check=True)
                MM(pa3[64:128, :], dwa3[:], ra2cd[:],
                   tile_position=(0, 64), skip_group_check=True)
                ablk = rhp.tile([128, 128], BF16, tag="rn")
                nc.scalar.activation(ablk[:], pa3[:], AF.Sigmoid, bias=ba3q[:, 0:1])
                if q == 0:
                    dbg("a0", ablk[:])
                axb = rhp.tile([128, 128], BF16, tag="rn")
                nc.vector.tensor_mul(axb[:], ablk[:], xq[:])
                nc.vector.tensor_reduce(xagg[:, q:q + 1], axb[:],
                                        mybir.AxisListType.X, ALU.add)

            # global MLP (fp32)
            if ABLATE < 8:
                return
            XaT = smallp.tile([F, GPC], FP32, tag="XaT")
            for m in range(4):
                nc.vector.tensor_copy(XaT[:, 8 * m:8 * (m + 1)],
                                      xagg[32 * m:32 * (m + 1), :])
            g1p = psmall.tile([H, GPC], FP32, tag="ps_small")
            MM(g1p[:], wgx[:], XaT[:], start=True, stop=False)
            MM(g1p[:], wgu[:], u_own[:], start=False, stop=True)
            g1 = smallp.tile([H, GPC], FP32, tag="g1")
            nc.scalar.activation(g1[:], g1p[:], AF.Relu, bias=bg1[:, 0:1])
            g2p = psmall.tile([H, GPC], FP32, tag="ps_small")
            MM(g2p[:], wg2[:], g1[:])
            g2 = smallp.tile([H, GPC], FP32, tag="g1")
            nc.scalar.activation(g2[:], g2p[:], AF.Relu, bias=bg2[:, 0:1])
            g3p = psmall.tile([F, GPC], FP32, tag="ps_small")
            MM(g3p[:], wg3[:], g2[:])
            unew = upool.tile([F, GPC], FP32, tag="u")
            nc.scalar.activation(unew[:], g3p[:], AF.Identity, bias=bg3[:, 0:1])
            uT[s] = unew
            dbg("u0p", unew[:])
            _dbg_done[0] = True

        for step in range(NSTEPS):
            gnn_pass(0)
            gnn_pass(1)
            o1p = psmall.tile([H, GPC], FP32, tag="ps_small")
            MM(o1p[:], wo1[:], uT[0][:], start=True, stop=False)
            MM(o1p[:], wo2[:], uT[1][:], start=False, stop=True)
            o1 = smallp.tile([H, GPC], FP32, tag="g1")
            nc.scalar.activation(o1[:], o1p[:], AF.Relu, bias=bo1[:, 0:1])
            o2p = psmall.tile([H, GPC], FP32, tag="ps_small")
            MM(o2p[:], wo_2[:], o1[:])
            o2 = smallp.tile([H, GPC], FP32, tag="g1")
            nc.scalar.activation(o2[:], o2p[:], AF.Relu, bias=bo2[:, 0:1])
            o3p = psmall.tile([F, GPC], FP32, tag="ps_small")
            MM(o3p[:], wo_3[:], o2[:])
            oT = smallp.tile([F, GPC], FP32, tag="oT")
            nc.scalar.activation(oT[:], o3p[:], AF.Identity, bias=bo3[:, 0:1])
            nc.sync.dma_start(out_d[F * step:F * (step + 1), :], oT[:])

    nc.compile()
    return nc


_NC = None


def _get_nc():
    global _NC
    if _NC is None:
        _NC = _build_graph()
    return _NC


# ----------------------------------------------------------------------------
# Host-side prep
# ----------------------------------------------------------------------------

def _diag2(w):
    k, m = w.shape
    out = np.zeros((2 * k, 2 * m), np.float32)
    out[:k, :m] = w
    out[k:, m:] = w
    return out


def _dup(w):
    # stack two copies along partitions
    return np.concatenate([w, w], axis=0)


def _prep_weights(edge_mlp, node_mlp, att_mlp, glob_mlp, out_mlp):
    ws = {}
    (W1, b1), (W2, b2), (W3, b3) = [(np.asarray(w, np.float32),
                                     np.asarray(b, np.float32)) for w, b in edge_mlp]
    # edge input layout: [xdiff(32) | u_oth-diff(32, zero) | e(32) | u_own(32)]
    w1x2 = np.zeros((128, 2 * H), np.float32)
    w1x2[0:32, 0:H] = W1[0:32]
    w1x2[32:64, H:2 * H] = W1[0:32]
    w1x2[64:96, 0:H] = W1[0:32]
    w1x2[96:128, H:2 * H] = W1[0:32]
    ws["w1x2"] = w1x2.astype(npbf16)
    ws["dw1e"] = _dup(_diag2(W1[64:96])).astype(npbf16)[:, :]
    # _dup(_diag2(.)) gives [128,128]? diag2 of [32,64] -> [64,128]; dup -> [128,128]
    ws["dw2"] = _diag2(W2).astype(npbf16)
    ws["dw3"] = _diag2(W3).astype(npbf16)
    ws["w3s2"] = _dup(W3).astype(npbf16)
    ws["w1u"] = W1[96:128].astype(np.float32)
    ws["b1"] = b1.reshape(H, 1).astype(np.float32)
    ws["b2p"] = np.concatenate([b2, b2]).reshape(128, 1).astype(np.float32)
    ws["b3q"] = np.concatenate([b3] * 4).reshape(128, 1).astype(np.float32)

    (Wn1, bn1), (Wn2, bn2), (Wn3, bn3) = [(np.asarray(w, np.float32),
                                           np.asarray(b, np.float32)) for w, b in node_mlp]
    # node input layout: [x(32) | u_oth(32) | eagg(32) | u_own(32)]
    ws["dwnx"] = _dup(_diag2(Wn1[0:32])).astype(npbf16)
    ws["dwne"] = _dup(_diag2(Wn1[64:96])).astype(npbf16)
    ws["dwn2"] = _diag2(Wn2).astype(npbf16)
    ws["dwn3"] = _diag2(Wn3).astype(npbf16)
    ws["wnuo"] = Wn1[32:64].astype(np.float32)
    ws["wnuu"] = Wn1[96:128].astype(np.float32)
    ws["bn1"] = bn1.reshape(H, 1).astype(np.float32)
    ws["bn2p"] = np.concatenate([bn2, bn2]).reshape(128, 1).astype(np.float32)
    ws["bn3q"] = np.concatenate([bn3] * 4).reshape(128, 1).astype(np.float32)

    (Wa1, ba1), (Wa2, ba2), (Wa3, ba3) = [(np.asarray(w, np.float32),
                                           np.asarray(b, np.float32)) for w, b in att_mlp]
    ws["dwax"] = _dup(_diag2(Wa1[0:32])).astype(npbf16)
    ws["dwa2"] = _diag2(Wa2).astype(npbf16)
    ws["dwa3"] = _diag2(Wa3).astype(npbf16)
    ws["wau"] = Wa1[32:64].astype(np.float32)
    ws["ba1"] = ba1.reshape(H, 1).astype(np.float32)
    ws["ba2p"] = np.concatenate([ba2, ba2]).reshape(128, 1).astype(np.float32)
    ws["ba3q"] = np.concatenate([ba3] * 4).reshape(128, 1).astype(np.float32)

    (Wg1, bg1), (Wg2, bg2), (Wg3, bg3) = [(np.asarray(w, np.float32),
                                           np.asarray(b, np.float32)) for w, b in glob_mlp]
    ws["wgx"] = Wg1[0:32].astype(np.float32)
    ws["wgu"] = Wg1[32:64].astype(np.float32)
    ws["wg2"] = Wg2.astype(np.float32)
    ws["wg3"] = Wg3.astype(np.float32)
    ws["bg1"] = bg1.reshape(H, 1).astype(np.float32)
    ws["bg2"] = bg2.reshape(H, 1).astype(np.float32)
    ws["bg3"] = bg3.reshape(F, 1).astype(np.float32)

    (Wo1, bo1), (Wo2, bo2), (Wo3, bo3) = [(np.asarray(w, np.float32),
                                           np.asarray(b, np.float32)) for w, b in out_mlp]
    ws["wo1"] = Wo1[0:32].astype(np.float32)
    ws["wo2"] = Wo1[32:64].astype(np.float32)
    ws["wo_2"] = Wo2.astype(np.float32)
    ws["wo_3"] = Wo3.astype(np.float32)
    ws["bo1"] = bo1.reshape(H, 1).astype(np.float32)
    ws["bo2"] = bo2.reshape(H, 1).astype(np.float32)
    ws["bo3"] = bo3.reshape(F, 1).astype(np.float32)

    return ws


def _prep_set(x, ei, e, u, core):
    g0 = GPC * core
    nsl = slice(NPG * g0, NPG * (g0 + GPC))
    esl = slice(EPG * g0, EPG * (g0 + GPC))
    xc = np.asarray(x[nsl], np.float32)        # [4096, 32]
    ec = np.asarray(e[esl], np.float32)        # [32768, 32]
    src = np.asarray(ei[0][esl], np.int64)
    dst = np.asarray(ei[1][esl], np.int64)

    xh = np.zeros((NQ * 128, 128), np.float32)
    ehm = np.zeros((NQ * 2 * 128, 512), np.float32)
    dsT = np.zeros((GPC * 128, EPG), np.float32)
    dem = np.zeros((GPC * 128, EPG), np.float32)
    for g in range(GPC):
        q, m = g // 4, g % 4
        xg = xc[NPG * g:NPG * (g + 1)]         # [128, 32]
        xh[128 * q + 32 * m:128 * q + 32 * (m + 1), :] = xg.T
        eg = ec[EPG * g:EPG * (g + 1)]         # [1024, 32]
        for t in range(2):
            i = 2 * q + t
            ehm[128 * i + 32 * m:128 * i + 32 * (m + 1), :] = \
                eg[512 * t:512 * (t + 1)].T
        sg = (src[EPG * g:EPG * (g + 1)] - NPG * (g0 + g)).astype(np.int64)
        dg = (dst[EPG * g:EPG * (g + 1)] - NPG * (g0 + g)).astype(np.int64)
        blk = dsT[128 * g:128 * (g + 1), :]
        np.add.at(blk, (dg, np.arange(EPG)), 1.0)
        np.add.at(blk, (sg, np.arange(EPG)), -1.0)
        dblk = dem[128 * g:128 * (g + 1), :]
        # D edge-major: per 128-edge chunk, rows = local edge, cols = dst node
        for c in range(8):
            sub = np.zeros((128, 128), np.float32)
            sub[np.arange(128), dg[128 * c:128 * (c + 1)]] = 1.0
            dblk[:, 128 * c:128 * (c + 1)] = sub

    uc = np.asarray(u[g0:g0 + GPC], np.float32)  # [32, 32]
    uT = np.zeros((F, GPC), np.float32)
    for g in range(GPC):
        uT[:, _SLOT[g]] = uc[g]
    return (xh.astype(npbf16), ehm.astype(npbf16),
            dsT.astype(npfp8), dem.astype(npfp8), uT)


def _build_in_maps(inputs):
    ws = _prep_weights(inputs["edge_mlp"], inputs["node_mlp"], inputs["att_mlp"],
                       inputs["glob_mlp"], inputs["out_mlp"])
    in_maps = []
    for core in range(NCORES):
        m = dict(ws)
        for s, (xk, eik, ek, uk) in enumerate(
                [("x1", "edge_index1", "e1", "u1"),
                 ("x2", "edge_index2", "e2", "u2")]):
            xh, ehm, dsT, dem, uT = _prep_set(
                np.asarray(inputs[xk]), np.asarray(inputs[eik]),
                np.asarray(inputs[ek]), np.asarray(inputs[uk]), core)
            m[f"xh{s}"] = xh
            m[f"eh{s}"] = ehm
            m[f"dsT{s}"] = dsT
            m[f"de{s}"] = dem
            m[f"uT{s}"] = uT
            b3 = np.asarray(inputs["edge_mlp"][2][1], np.float32)
            g0 = GPC * core
            dstv = np.asarray(inputs[eik][1], np.int64)
            ac = np.zeros((NQ * 128, 128), np.float32)
            for g in range(GPC):
                q, mm_ = g // 4, g % 4
                dg = dstv[EPG * (g0 + g):EPG * (g0 + g + 1)] - NPG * (g0 + g)
                indeg = np.bincount(dg, minlength=NPG).astype(np.float32)
                ac[128 * q:128 * (q + 1), 32 * mm_:32 * (mm_ + 1)] = \
                    np.outer(indeg, b3)
            m[f"aggcorr{s}"] = ac
        in_maps.append(m)
    return in_maps


def _execute(inputs, trace=False):
    nc = _get_nc()
    in_maps = _build_in_maps(inputs)
    res = run_bass_kernel_spmd(nc, in_maps, core_ids=list(range(NCORES)),
                               trace=trace)
    full = np.zeros((NSTEPS, B, F), np.float32)
    for core in range(NCORES):
        o = np.asarray(res.results[core]["out"], np.float32).reshape(NSTEPS, F, GPC)
        for g in range(GPC):
            full[:, GPC * core + g, :] = o[:, :, _SLOT[g]]
    return full, res


def kernel(**inputs):
    out, _ = _execute(inputs, trace=False)
    return out


# revision 23
# speedup vs baseline: 1.4623x; 1.1532x over previous
import sys
import os

sys.path.insert(0, "/opt/trn_rl_repo")

from contextlib import ExitStack

import numpy as np
import ml_dtypes

import concourse.bass as bass
import concourse.tile as tile
from concourse import bacc, mybir
from concourse.bass_utils import run_bass_kernel_spmd

BF16 = mybir.dt.float16  # 16-bit compute dtype (fp16: more mantissa than bf16)
FP32 = mybir.dt.float32
FP8 = mybir.dt.float8e4
AF = mybir.ActivationFunctionType
ALU = mybir.AluOpType
npbf16 = np.float16
npfp8 = ml_dtypes.float8_e4m3

ABLATE = int(os.environ.get("ABLATE", "99"))
KDEBUG = int(os.environ.get("KDEBUG", "0"))
NCORES = 8
B = 256          # graphs total
NPG = 128        # nodes per graph
EPG = 1024       # edges per graph
GPC = B // NCORES  # 32 graphs per core
NQ = GPC // 4      # 8 quads per core
F = 32
H = 64
NSTEPS = 2


def _slotcol(g):
    # column order of graphs in uT / XaT / beff tensors (m-major)
    q, m = g // 4, g % 4
    return 8 * m + q


_SLOT = [_slotcol(g) for g in range(GPC)]
_INV_SLOT = [0] * GPC
for _g, _s in enumerate(_SLOT):
    _INV_SLOT[_s] = _g


# ----------------------------------------------------------------------------
# Device graph
# ----------------------------------------------------------------------------

def _build_graph():
    nc = bacc.Bacc(
        "TRN2",
        target_bir_lowering=False,
        debug=False,
        enable_asserts=False,
        num_devices=NCORES,
    )

    def din(name, shape, dt):
        return nc.dram_tensor(name, shape, dt, kind="ExternalInput").ap()

    # per-set big inputs
    xh_d = [din(f"xh{s}", [NQ * 128, 128], BF16) for s in range(2)]
    eh_d = [din(f"eh{s}", [NQ * 2 * 128, 512], BF16) for s in range(2)]
    dsT_d = [din(f"dsT{s}", [GPC * 128, EPG], FP8) for s in range(2)]
    de_d = [din(f"de{s}", [GPC * 128, EPG], FP8) for s in range(2)]
    uT_d = [din(f"uT{s}", [F, GPC], FP32) for s in range(2)]

    # weights (bf16 compute path)
    w1x2_d = din("w1x2", [128, 2 * H], BF16)  # W1 x-part in half-band layout
    dw1e_d = din("dw1e", [128, 128], BF16)   # diag2(W1 e-part), dup'd rows 0-63/64-127
    dw2_d = din("dw2", [128, 128], BF16)     # diag2(W2)
    dw3_d = din("dw3", [128, H], BF16)       # diag2(W3)
    dwnx_d = din("dwnx", [128, 128], BF16)   # diag2(Wn x-part) dup'd
    dwne_d = din("dwne", [128, 128], BF16)   # diag2(Wn eagg-part) dup'd
    dwn2_d = din("dwn2", [128, 128], BF16)
    dwn3_d = din("dwn3", [128, H], BF16)
    dwax_d = din("dwax", [128, 128], BF16)   # diag2(Wa x-part) dup'd
    dwa2_d = din("dwa2", [128, 128], BF16)
    dwa3_d = din("dwa3", [128, H], BF16)
    ident_d = din("ident", [128, 128], BF16)
    aggcorr_d = [din(f"aggcorr{s}", [NQ * 128, 128], FP32) for s in range(2)]

    # fp32 u-path weights
    w1u_d = din("w1u", [F, H], FP32)
    wnuo_d = din("wnuo", [F, H], FP32)
    wnuu_d = din("wnuu", [F, H], FP32)
    wau_d = din("wau", [F, H], FP32)
    wgx_d = din("wgx", [F, H], FP32)
    wgu_d = din("wgu", [F, H], FP32)
    wg2_d = din("wg2", [H, H], FP32)
    wg3_d = din("wg3", [H, F], FP32)
    wo1_d = din("wo1", [F, H], FP32)
    wo2_d = din("wo2", [F, H], FP32)
    wo_2_d = din("wo_2", [H, H], FP32)
    wo_3_d = din("wo_3", [H, F], FP32)

    # biases
    b1_d = din("b1", [H, 1], FP32)
    b2p_d = din("b2p", [128, 1], FP32)
    b3q_d = din("b3q", [128, 1], FP32)
    bn1_d = din("bn1", [H, 1], FP32)
    bn2p_d = din("bn2p", [128, 1], FP32)
    bn3q_d = din("bn3q", [128, 1], FP32)
    ba1_d = din("ba1", [H, 1], FP32)
    ba2p_d = din("ba2p", [128, 1], FP32)
    ba3q_d = din("ba3q", [128, 1], FP32)
    bg1_d = din("bg1", [H, 1], FP32)
    bg2_d = din("bg2", [H, 1], FP32)
    bg3_d = din("bg3", [F, 1], FP32)
    bo1_d = din("bo1", [H, 1], FP32)
    bo2_d = din("bo2", [H, 1], FP32)
    bo3_d = din("bo3", [F, 1], FP32)

    out_d = nc.dram_tensor("out", [NSTEPS * F, GPC], FP32, kind="ExternalOutput").ap()
    dbg_d = {}
    if KDEBUG:
        for nm, shp in [("pb1", [128, 16]), ("y0", [128, H]), ("rh1", [128, 512]),
                        ("rh2", [128, 512]), ("enew", [128, 512]), ("tsb0", [128, 128]),
                        ("eagg0", [128, 128]), ("xnew0", [128, 128]), ("a0", [128, 128]),
                        ("u0p", [F, GPC])]:
            dbg_d[nm] = nc.dram_tensor("dbg_" + nm, shp, FP32, kind="ExternalOutput").ap()

    with ExitStack() as ctx:
        tc = ctx.enter_context(tile.TileContext(nc))

        const = ctx.enter_context(tc.tile_pool(name="const", bufs=1))
        persist = ctx.enter_context(tc.tile_pool(name="persist", bufs=1))
        pbig = ctx.enter_context(tc.tile_pool(name="pbig", bufs=5, space="PSUM"))
        pagg = ctx.enter_context(tc.tile_pool(name="pagg", bufs=1, space="PSUM"))
        psmall = ctx.enter_context(tc.tile_pool(name="psmall", bufs=2, space="PSUM"))
        rhp = ctx.enter_context(tc.tile_pool(name="rhp", bufs=8))
        tsbp = ctx.enter_context(tc.tile_pool(name="tsbp", bufs=3))
        ysbp = ctx.enter_context(tc.tile_pool(name="ysbp", bufs=8))
        eaggp = ctx.enter_context(tc.tile_pool(name="eaggp", bufs=4))
        smallp = ctx.enter_context(tc.tile_pool(name="smallp", bufs=4))
        upool = ctx.enter_context(tc.tile_pool(name="upool", bufs=4))

        def cload(name, dram, shape, dt):
            t = const.tile(shape, dt, tag=name)
            nc.sync.dma_start(t[:], dram)
            return t

        w1x2 = cload("w1x2", w1x2_d, [128, 2 * H], BF16)
        dw1e = cload("dw1e", dw1e_d, [128, 128], BF16)
        dw2 = cload("dw2", dw2_d, [128, 128], BF16)
        dw3 = cload("dw3", dw3_d, [128, H], BF16)
        dwnx = cload("dwnx", dwnx_d, [128, 128], BF16)
        dwne = cload("dwne", dwne_d, [128, 128], BF16)
        dwn2 = cload("dwn2", dwn2_d, [128, 128], BF16)
        dwn3 = cload("dwn3", dwn3_d, [128, H], BF16)
        dwax = cload("dwax", dwax_d, [128, 128], BF16)
        dwa2 = cload("dwa2", dwa2_d, [128, 128], BF16)
        dwa3 = cload("dwa3", dwa3_d, [128, H], BF16)
        ident = cload("ident", ident_d, [128, 128], BF16)

        w1u = cload("w1u", w1u_d, [F, H], FP32)
        wnuo = cload("wnuo", wnuo_d, [F, H], FP32)
        wnuu = cload("wnuu", wnuu_d, [F, H], FP32)
        wau = cload("wau", wau_d, [F, H], FP32)
        wgx = cload("wgx", wgx_d, [F, H], FP32)
        wgu = cload("wgu", wgu_d, [F, H], FP32)
        wg2 = cload("wg2", wg2_d, [H, H], FP32)
        wg3 = cload("wg3", wg3_d, [H, F], FP32)
        wo1 = cload("wo1", wo1_d, [F, H], FP32)
        wo2 = cload("wo2", wo2_d, [F, H], FP32)
        wo_2 = cload("wo_2", wo_2_d, [H, H], FP32)
        wo_3 = cload("wo_3", wo_3_d, [H, F], FP32)

        b1 = cload("b1", b1_d, [H, 1], FP32)
        b2p = cload("b2p", b2p_d, [128, 1], FP32)
        b3q = cload("b3q", b3q_d, [128, 1], FP32)
        bn1 = cload("bn1", bn1_d, [H, 1], FP32)
        bn2p = cload("bn2p", bn2p_d, [128, 1], FP32)
        bn3q = cload("bn3q", bn3q_d, [128, 1], FP32)
        ba1 = cload("ba1", ba1_d, [H, 1], FP32)
        ba2p = cload("ba2p", ba2p_d, [128, 1], FP32)
        ba3q = cload("ba3q", ba3q_d, [128, 1], FP32)
        bg1 = cload("bg1", bg1_d, [H, 1], FP32)
        bg2 = cload("bg2", bg2_d, [H, 1], FP32)
        bg3 = cload("bg3", bg3_d, [F, 1], FP32)
        bo1 = cload("bo1", bo1_d, [H, 1], FP32)
        bo2 = cload("bo2", bo2_d, [H, 1], FP32)
        bo3 = cload("bo3", bo3_d, [F, 1], FP32)

        # persistent state tiles
        xh = [[persist.tile([128, 128], BF16, tag=f"xh{s}_{q}", name=f"xh{s}_{q}") for q in range(NQ)]
              for s in range(2)]
        eh = [[persist.tile([128, 512], BF16, tag=f"eh{s}_{i}", name=f"eh{s}_{i}") for i in range(NQ * 2)]
              for s in range(2)]
        dsT = [[persist.tile([128, EPG], FP8, tag=f"dsT{s}_{g}", name=f"dsT{s}_{g}") for g in range(GPC)]
               for s in range(2)]
        de = [[persist.tile([128, EPG], FP8, tag=f"de{s}_{g}", name=f"de{s}_{g}") for g in range(GPC)]
              for s in range(2)]
        acorr = [[persist.tile([128, 128], FP32, tag=f"ac{s}_{q}", name=f"ac{s}_{q}")
                  for q in range(NQ)] for s in range(2)]
        def ldma(dst, srcap):
            nc.sync.dma_start(dst, srcap)

        for s in range(2):
            for q in range(NQ):
                ldma(acorr[s][q][:], aggcorr_d[s][128 * q:128 * (q + 1), :])
            for q in range(NQ):
                ldma(xh[s][q][:], xh_d[s][128 * q:128 * (q + 1), :])
            for i in range(NQ * 2):
                ldma(eh[s][i][:], eh_d[s][128 * i:128 * (i + 1), :])
            for g in range(GPC):
                ldma(dsT[s][g][:], dsT_d[s][128 * g:128 * (g + 1), :])
                ldma(de[s][g][:], de_d[s][128 * g:128 * (g + 1), :])

        uT = []
        for s in range(2):
            t = upool.tile([F, GPC], FP32, tag="u")
            nc.sync.dma_start(t[:], uT_d[s])
            uT.append(t)

        MM = nc.tensor.matmul

        def pairbias(beff, tag):
            # beff [64, GPC] (slot order) -> pb [128, 16] (pair order)
            pb = smallp.tile([128, 16], FP32, tag=tag)
            src = beff[:, 0:32].rearrange("f (b r) -> f b r", b=2)
            dst_hi = pb[0:64, :].rearrange("f (a b) -> f b a", a=8)
            nc.vector.tensor_copy(dst_hi, src[:, :, 0:8])
            src2 = beff[:, 0:32].rearrange("f (b r) -> f b r", b=2)
            dst_lo = pb[64:128, :].rearrange("f (a b) -> f b a", a=8)
            nc.vector.tensor_copy(dst_lo, src2[:, :, 8:16])
            return pb

        def beff_mm(wlist, ulist, bias, tag):
            # returns [64, GPC] fp32 = sum_i wlist[i].T @ ulist[i] + bias
            ps = psmall.tile([H, GPC], FP32, tag="ps_small")
            for i, (w, u) in enumerate(zip(wlist, ulist)):
                MM(ps[:], w[:], u[:], start=(i == 0), stop=(i == len(wlist) - 1))
            be = smallp.tile([H, GPC], FP32, tag=tag)
            nc.scalar.activation(be[:], ps[:], AF.Identity, bias=bias[:, 0:1])
            return be

        _dbg_done = [False]

        def dbg(nm, ap):
            if KDEBUG and not _dbg_done[0] and nm in dbg_d:
                t = smallp.tile(list(ap.shape), FP32, tag="dbgt", name="dbgt")
                nc.vector.tensor_copy(t[:], ap)
                nc.sync.dma_start(dbg_d[nm], t[:])

        def gnn_pass(s):
            u_own, u_oth = uT[s], uT[1 - s]
            if ABLATE < 2:
                return
            be1 = beff_mm([w1u], [u_own], b1, "be1")
            pb1 = pairbias(be1, "pb1")
            ben = beff_mm([wnuo, wnuu], [u_oth, u_own], bn1, "ben")
            pbn = pairbias(ben, "pbn")
            bea = beff_mm([wau], [u_own], ba1, "bea")
            pba = pairbias(bea, "pba")
            dbg("pb1", pb1[:])

            xagg = smallp.tile([128, NQ], FP32, tag="xagg")

            if ABLATE < 3:
                return
            for q in range(NQ):
                xq = xh[s][q]
                # y_m = x_g @ W1x  (node-major), per quad member
                ys = []
                for m in range(4):
                    half = 64 * (m // 2)
                    yp = psmall.tile([128, H], FP32, tag="ps_small")
                    MM(yp[:], xq[half:half + 64, :],
                       w1x2[half:half + 64, H * (m % 2):H * (m % 2) + H])
                    yt = ysbp.tile([128, H], BF16, tag="ysb")
                    nc.vector.tensor_copy(yt[:], yp[:])
                    ys.append(yt)
                    if m == 0 and q == 0:
                        dbg("y0", yt[:])

                agg = None
                if not os.environ.get("NOSCAT"):
                    agg = pagg.tile([128, 128], FP32, tag="agg")

                if ABLATE < 4:
                    continue
                for t in range(2):
                    ehb = eh[s][2 * q + t]
                    cs = slice(512 * t, 512 * (t + 1))
                    h1ab = pbig.tile([128, 512], FP32, tag="hps")
                    MM(h1ab[:], dw1e[0:64, :], ehb[0:64, :],
                       start=True, stop=False, skip_group_check=True)
                    MM(h1ab[0:64, :], ys[0][:], dsT[s][4 * q][:, cs],
                       start=False, stop=False, tile_position=(0, 0),
                       skip_group_check=True)
                    MM(h1ab[64:128, :], ys[1][:], dsT[s][4 * q + 1][:, cs],
                       start=False, stop=True, tile_position=(0, 64),
                       skip_group_check=True)
                    h1cd = pbig.tile([128, 512], FP32, tag="hps")
                    MM(h1cd[:], dw1e[64:128, :], ehb[64:128, :],
                       start=True, stop=False, skip_group_check=True)
                    MM(h1cd[0:64, :], ys[2][:], dsT[s][4 * q + 2][:, cs],
                       start=False, stop=False, tile_position=(0, 0),
                       skip_group_check=True)
                    MM(h1cd[64:128, :], ys[3][:], dsT[s][4 * q + 3][:, cs],
                       start=False, stop=True, tile_position=(0, 64),
                       skip_group_check=True)

                    rh1ab = rhp.tile([128, 512], BF16, tag="rh")
                    nc.scalar.activation(rh1ab[:], h1ab[:], AF.Relu,
                                         bias=pb1[:, 2 * q:2 * q + 1])
                    rh1cd = rhp.tile([128, 512], BF16, tag="rh")
                    nc.vector.tensor_scalar(rh1cd[:], h1cd[:],
                                            pb1[:, 2 * q + 1:2 * q + 2], 0.0,
                                            op0=ALU.add, op1=ALU.max)

                    if ABLATE < 5:
                        continue
                    if q == 0 and t == 0:
                        dbg("rh1", rh1ab[:])
                    h2ab = pbig.tile([128, 512], FP32, tag="hps")
                    MM(h2ab[:], dw2[:], rh1ab[:])
                    h2cd = pbig.tile([128, 512], FP32, tag="hps")
                    MM(h2cd[:], dw2[:], rh1cd[:])

                    rh2ab = rhp.tile([128, 512], BF16, tag="rh")
                    nc.scalar.activation(rh2ab[:], h2ab[:], AF.Relu,
                                         bias=b2p[:, 0:1])
                    rh2cd = rhp.tile([128, 512], BF16, tag="rh")
                    nc.vector.tensor_scalar(rh2cd[:], h2cd[:],
                                            b2p[:, 0:1], 0.0,
                                            op0=ALU.add, op1=ALU.max)

                    if q == 0 and t == 0:
                        dbg("rh2", rh2ab[:])
                    eps = pbig.tile([128, 512], FP32, tag="hps")
                    MM(eps[0:64, :], dw3[:], rh2ab[:],
                       tile_position=(0, 0), skip_group_check=True)
                    MM(eps[64:128, :], dw3[:], rh2cd[:],
                       tile_position=(0, 64), skip_group_check=True)
                    # e_new written back in place into the e-home block
                    nc.scalar.activation(ehb[:], eps[:], AF.Identity,
                                         bias=b3q[:, 0:1])

                    if ABLATE < 6:
                        continue
                    if q == 0 and t == 0:
                        dbg("enew", ehb[:])
                    # e_new edge-major (no b3; corrected at agg evict):
                    # lhsT = pair-stacked rh2 chunk (K=128), rhs = diag2(W3)
                    epsE = pbig.tile([128, 512], FP32, tag="hps")
                    for cc in range(4):
                        MM(epsE[:, 128 * cc:128 * cc + 64],
                           rh2ab[:, 128 * cc:128 * (cc + 1)], dw3[:],
                           start=(cc == 0), stop=False, skip_group_check=True)
                        MM(epsE[:, 128 * cc + 64:128 * (cc + 1)],
                           rh2cd[:, 128 * cc:128 * (cc + 1)], dw3[:],
                           start=False, stop=(cc == 3), skip_group_check=True)
                    tsb = tsbp.tile([128, 512], BF16, tag="tsb")
                    nc.vector.tensor_copy(tsb[:], epsE[:])
                    for cc in range(4):
                        c = 4 * t + cc
                        for m in range(4):
                            mo = 64 * (m // 2) + 32 * (m % 2)
                            MM(agg[:, 32 * m:32 * (m + 1)],
                               de[s][4 * q + m][:, 128 * c:128 * (c + 1)],
                               tsb[:, 128 * cc + mo:128 * cc + mo + 32],
                               start=(c == 0 and m == 0),
                               stop=(c == 7 and m == 3),
                               skip_group_check=True)

                if ABLATE < 6 or os.environ.get("NOSCAT"):
                    continue
                eagg_nm = eaggp.tile([128, 128], BF16, tag="eagg_nm")
                nc.vector.tensor_add(eagg_nm[:], agg[:], acorr[s][q][:])
                tps = pbig.tile([128, 128], BF16, tag="hps")
                nc.tensor.transpose(tps[:], eagg_nm[:], ident[:])
                eagg = eaggp.tile([128, 128], BF16, tag="eagg")
                nc.vector.tensor_copy(eagg[:], tps[:])
                if q == 0:
                    dbg("eagg0", eagg[:])

                if ABLATE < 7:
                    continue
                # node MLP
                pn_ab = psmall.tile([128, 128], FP32, tag="ps_small")
                MM(pn_ab[:], dwnx[0:64, :], xq[0:64, :], start=True, stop=False)
                MM(pn_ab[:], dwne[0:64, :], eagg[0:64, :], start=False, stop=True)
                pn_cd = psmall.tile([128, 128], FP32, tag="ps_small")
                MM(pn_cd[:], dwnx[64:128, :], xq[64:128, :], start=True, stop=False)
                MM(pn_cd[:], dwne[64:128, :], eagg[64:128, :], start=False, stop=True)
                rn1ab = rhp.tile([128, 128], BF16, tag="rn")
                nc.scalar.activation(rn1ab[:], pn_ab[:], AF.Relu,
                                     bias=pbn[:, 2 * q:2 * q + 1])
                rn1cd = rhp.tile([128, 128], BF16, tag="rn")
                nc.vector.tensor_scalar(rn1cd[:], pn_cd[:],
                                        pbn[:, 2 * q + 1:2 * q + 2], 0.0,
                                        op0=ALU.add, op1=ALU.max)
                pn2ab = psmall.tile([128, 128], FP32, tag="ps_small")
                MM(pn2ab[:], dwn2[:], rn1ab[:])
                pn2cd = psmall.tile([128, 128], FP32, tag="ps_small")
                MM(pn2cd[:], dwn2[:], rn1cd[:])
                rn2ab = rhp.tile([128, 128], BF16, tag="rn")
                nc.scalar.activation(rn2ab[:], pn2ab[:], AF.Relu, bias=bn2p[:, 0:1])
                rn2cd = rhp.tile([128, 128], BF16, tag="rn")
                nc.vector.tensor_scalar(rn2cd[:], pn2cd[:], bn2p[:, 0:1], 0.0,
                                        op0=ALU.add, op1=ALU.max)
                px = psmall.tile([128, 128], FP32, tag="ps_small")
                MM(px[0:64, :], dwn3[:], rn2ab[:],
                   tile_position=(0, 0), skip_group_check=True)
                MM(px[64:128, :], dwn3[:], rn2cd[:],
                   tile_position=(0, 64), skip_group_check=True)
                # x_new in place
                nc.scalar.activation(xq[:], px[:], AF.Identity, bias=bn3q[:, 0:1])

                if q == 0:
                    dbg("xnew0", xq[:])
                # attention MLP
                pa_ab = psmall.tile([128, 128], FP32, tag="ps_small")
                MM(pa_ab[:], dwax[0:64, :], xq[0:64, :])
                pa_cd = psmall.tile([128, 128], FP32, tag="ps_small")
                MM(pa_cd[:], dwax[64:128, :], xq[64:128, :])
                ra1ab = rhp.tile([128, 128], BF16, tag="rn")
                nc.scalar.activation(ra1ab[:], pa_ab[:], AF.Relu,
                                     bias=pba[:, 2 * q:2 * q + 1])
                ra1cd = rhp.tile([128, 128], BF16, tag="rn")
                nc.vector.tensor_scalar(ra1cd[:], pa_cd[:],
                                        pba[:, 2 * q + 1:2 * q + 2], 0.0,
                                        op0=ALU.add, op1=ALU.max)
                pa2ab = psmall.tile([128, 128], FP32, tag="ps_small")
                MM(pa2ab[:], dwa2[:], ra1ab[:])
                pa2cd = psmall.tile([128, 128], FP32, tag="ps_small")
                MM(pa2cd[:], dwa2[:], ra1cd[:])
                ra2ab = rhp.tile([128, 128], BF16, tag="rn")
                nc.scalar.activation(ra2ab[:], pa2ab[:], AF.Relu, bias=ba2p[:, 0:1])
                ra2cd = rhp.tile([128, 128], BF16, tag="rn")
                nc.vector.tensor_scalar(ra2cd[:], pa2cd[:], ba2p[:, 0:1], 0.0,
                                        op0=ALU.add, op1=ALU.max)
                pa3 = psmall.tile([128, 128], FP32, tag="ps_small")
                MM(pa3[0:64, :], dwa3[:], ra2ab[:],
                   tile_position=(0, 0), skip_group_check=True)
                MM(pa3[64:128, :], dwa3[:], ra2cd[:],
                   tile_position=(0, 64), skip_group_check=True)
                ablk = rhp.tile([128, 128], BF16, tag="rn")
                nc.scalar.activation(ablk[:], pa3[:], AF.Sigmoid, bias=ba3q[:, 0:1])
                if q == 0:
                    dbg("a0", ablk[:])
                axb = rhp.tile([128, 128], BF16, tag="rn")
                nc.vector.tensor_mul(axb[:], ablk[:], xq[:])
                nc.vector.tensor_reduce(xagg[:, q:q + 1], axb[:],
                                        mybir.AxisListType.X, ALU.add)

            # global MLP (fp32)
            if ABLATE < 8:
                return
            XaT = smallp.tile([F, GPC], FP32, tag="XaT")
            for m in range(4):
                nc.vector.tensor_copy(XaT[:, 8 * m:8 * (m + 1)],
                                      xagg[32 * m:32 * (m + 1), :])
            g1p = psmall.tile([H, GPC], FP32, tag="ps_small")
            MM(g1p[:], wgx[:], XaT[:], start=True, stop=False)
            MM(g1p[:], wgu[:], u_own[:], start=False, stop=True)
            g1 = smallp.tile([H, GPC], FP32, tag="g1")
            nc.scalar.activation(g1[:], g1p[:], AF.Relu, bias=bg1[:, 0:1])
            g2p = psmall.tile([H, GPC], FP32, tag="ps_small")
            MM(g2p[:], wg2[:], g1[:])
            g2 = smallp.tile([H, GPC], FP32, tag="g1")
            nc.scalar.activation(g2[:], g2p[:], AF.Relu, bias=bg2[:, 0:1])
            g3p = psmall.tile([F, GPC], FP32, tag="ps_small")
            MM(g3p[:], wg3[:], g2[:])
            unew = upool.tile([F, GPC], FP32, tag="u")
            nc.scalar.activation(unew[:], g3p[:], AF.Identity, bias=bg3[:, 0:1])
            uT[s] = unew
            dbg("u0p", unew[:])
            _dbg_done[0] = True

        for step in range(NSTEPS):
            gnn_pass(0)
            gnn_pass(1)
            o1p = psmall.tile([H, GPC], FP32, tag="ps_small")
            MM(o1p[:], wo1[:], uT[0][:], start=True, stop=False)
            MM(o1p[:], wo2[:], uT[1][:], start=False, stop=True)
            o1 = smallp.tile([H, GPC], FP32, tag="g1")
            nc.scalar.activation(o1[:], o1p[:], AF.Relu, bias=bo1[:, 0:1])
            o2p = psmall.tile([H, GPC], FP32, tag="ps_small")
            MM(o2p[:], wo_2[:], o1[:])
            o2 = smallp.tile([H, GPC], FP32, tag="g1")
            nc.scalar.activation(o2[:], o2p[:], AF.Relu, bias=bo2[:, 0:1])
            o3p = psmall.tile([F, GPC], FP32, tag="ps_small")
            MM(o3p[:], wo_3[:], o2[:])
            oT = smallp.tile([F, GPC], FP32, tag="oT")
            nc.scalar.activation(oT[:], o3p[:], AF.Identity, bias=bo3[:, 0:1])
            nc.sync.dma_start(out_d[F * step:F * (step + 1), :], oT[:])

    nc.compile()
    return nc


_NC = None


def _get_nc():
    global _NC
    if _NC is None:
        _NC = _build_graph()
    return _NC


# ----------------------------------------------------------------------------
# Host-side prep
# ----------------------------------------------------------------------------

def _diag2(w):
    k, m = w.shape
    out = np.zeros((2 * k, 2 * m), np.float32)
    out[:k, :m] = w
    out[k:, m:] = w
    return out


def _dup(w):
    # stack two copies along partitions
    return np.concatenate([w, w], axis=0)


def _prep_weights(edge_mlp, node_mlp, att_mlp, glob_mlp, out_mlp):
    ws = {}
    (W1, b1), (W2, b2), (W3, b3) = [(np.asarray(w, np.float32),
                                     np.asarray(b, np.float32)) for w, b in edge_mlp]
    # edge input layout: [xdiff(32) | u_oth-diff(32, zero) | e(32) | u_own(32)]
    w1x2 = np.zeros((128, 2 * H), np.float32)
    w1x2[0:32, 0:H] = W1[0:32]
    w1x2[32:64, H:2 * H] = W1[0:32]
    w1x2[64:96, 0:H] = W1[0:32]
    w1x2[96:128, H:2 * H] = W1[0:32]
    ws["w1x2"] = w1x2.astype(npbf16)
    ws["dw1e"] = _dup(_diag2(W1[64:96])).astype(npbf16)[:, :]
    # _dup(_diag2(.)) gives [128,128]? diag2 of [32,64] -> [64,128]; dup -> [128,128]
    ws["dw2"] = _diag2(W2).astype(npbf16)
    ws["dw3"] = _diag2(W3).astype(npbf16)
    ws["w1u"] = W1[96:128].astype(np.float32)
    ws["b1"] = b1.reshape(H, 1).astype(np.float32)
    ws["b2p"] = np.concatenate([b2, b2]).reshape(128, 1).astype(np.float32)
    ws["b3q"] = np.concatenate([b3] * 4).reshape(128, 1).astype(np.float32)

    (Wn1, bn1), (Wn2, bn2), (Wn3, bn3) = [(np.asarray(w, np.float32),
                                           np.asarray(b, np.float32)) for w, b in node_mlp]
    # node input layout: [x(32) | u_oth(32) | eagg(32) | u_own(32)]
    ws["dwnx"] = _dup(_diag2(Wn1[0:32])).astype(npbf16)
    ws["dwne"] = _dup(_diag2(Wn1[64:96])).astype(npbf16)
    ws["dwn2"] = _diag2(Wn2).astype(npbf16)
    ws["dwn3"] = _diag2(Wn3).astype(npbf16)
    ws["wnuo"] = Wn1[32:64].astype(np.float32)
    ws["wnuu"] = Wn1[96:128].astype(np.float32)
    ws["bn1"] = bn1.reshape(H, 1).astype(np.float32)
    ws["bn2p"] = np.concatenate([bn2, bn2]).reshape(128, 1).astype(np.float32)
    ws["bn3q"] = np.concatenate([bn3] * 4).reshape(128, 1).astype(np.float32)

    (Wa1, ba1), (Wa2, ba2), (Wa3, ba3) = [(np.asarray(w, np.float32),
                                           np.asarray(b, np.float32)) for w, b in att_mlp]
    ws["dwax"] = _dup(_diag2(Wa1[0:32])).astype(npbf16)
    ws["dwa2"] = _diag2(Wa2).astype(npbf16)
    ws["dwa3"] = _diag2(Wa3).astype(npbf16)
    ws["wau"] = Wa1[32:64].astype(np.float32)
    ws["ba1"] = ba1.reshape(H, 1).astype(np.float32)
    ws["ba2p"] = np.concatenate([ba2, ba2]).reshape(128, 1).astype(np.float32)
    ws["ba3q"] = np.concatenate([ba3] * 4).reshape(128, 1).astype(np.float32)

    (Wg1, bg1), (Wg2, bg2), (Wg3, bg3) = [(np.asarray(w, np.float32),
                                           np.asarray(b, np.float32)) for w, b in glob_mlp]
    ws["wgx"] = Wg1[0:32].astype(np.float32)
    ws["wgu"] = Wg1[32:64].astype(np.float32)
    ws["wg2"] = Wg2.astype(np.float32)
    ws["wg3"] = Wg3.astype(np.float32)
    ws["bg1"] = bg1.reshape(H, 1).astype(np.float32)
    ws["bg2"] = bg2.reshape(H, 1).astype(np.float32)
    ws["bg3"] = bg3.reshape(F, 1).astype(np.float32)

    (Wo1, bo1), (Wo2, bo2), (Wo3, bo3) = [(np.asarray(w, np.float32),
                                           np.asarray(b, np.float32)) for w, b in out_mlp]
    ws["wo1"] = Wo1[0:32].astype(np.float32)
    ws["wo2"] = Wo1[32:64].astype(np.float32)
    ws["wo_2"] = Wo2.astype(np.float32)
    ws["wo_3"] = Wo3.astype(np.float32)
    ws["ident"] = np.eye(128, dtype=npbf16)
    ws["bo1"] = bo1.reshape(H, 1).astype(np.float32)
    ws["bo2"] = bo2.reshape(H, 1).astype(np.float32)
    ws["bo3"] = bo3.reshape(F, 1).astype(np.float32)

    return ws


def _prep_set(x, ei, e, u, core):
    g0 = GPC * core
    nsl = slice(NPG * g0, NPG * (g0 + GPC))
    esl = slice(EPG * g0, EPG * (g0 + GPC))
    xc = np.asarray(x[nsl], np.float32)        # [4096, 32]
    ec = np.asarray(e[esl], np.float32)        # [32768, 32]
    src = np.asarray(ei[0][esl], np.int64)
    dst = np.asarray(ei[1][esl], np.int64)

    xh = np.zeros((NQ * 128, 128), np.float32)
    ehm = np.zeros((NQ * 2 * 128, 512), np.float32)
    dsT = np.zeros((GPC * 128, EPG), np.float32)
    dem = np.zeros((GPC * 128, EPG), np.float32)
    for g in range(GPC):
        q, m = g // 4, g % 4
        xg = xc[NPG * g:NPG * (g + 1)]         # [128, 32]
        xh[128 * q + 32 * m:128 * q + 32 * (m + 1), :] = xg.T
        eg = ec[EPG * g:EPG * (g + 1)]         # [1024, 32]
        for t in range(2):
            i = 2 * q + t
            ehm[128 * i + 32 * m:128 * i + 32 * (m + 1), :] = \
                eg[512 * t:512 * (t + 1)].T
        sg = (src[EPG * g:EPG * (g + 1)] - NPG * (g0 + g)).astype(np.int64)
        dg = (dst[EPG * g:EPG * (g + 1)] - NPG * (g0 + g)).astype(np.int64)
        blk = dsT[128 * g:128 * (g + 1), :]
        np.add.at(blk, (dg, np.arange(EPG)), 1.0)
        np.add.at(blk, (sg, np.arange(EPG)), -1.0)
        dblk = dem[128 * g:128 * (g + 1), :]
        # D edge-major: per 128-edge chunk, rows = local edge, cols = dst node
        for c in range(8):
            sub = np.zeros((128, 128), np.float32)
            sub[np.arange(128), dg[128 * c:128 * (c + 1)]] = 1.0
            dblk[:, 128 * c:128 * (c + 1)] = sub

    uc = np.asarray(u[g0:g0 + GPC], np.float32)  # [32, 32]
    uT = np.zeros((F, GPC), np.float32)
    for g in range(GPC):
        uT[:, _SLOT[g]] = uc[g]
    return (xh.astype(npbf16), ehm.astype(npbf16),
            dsT.astype(npfp8), dem.astype(npfp8), uT)


def _build_in_maps(inputs):
    ws = _prep_weights(inputs["edge_mlp"], inputs["node_mlp"], inputs["att_mlp"],
                       inputs["glob_mlp"], inputs["out_mlp"])
    in_maps = []
    for core in range(NCORES):
        m = dict(ws)
        for s, (xk, eik, ek, uk) in enumerate(
                [("x1", "edge_index1", "e1", "u1"),
                 ("x2", "edge_index2", "e2", "u2")]):
            xh, ehm, dsT, dem, uT = _prep_set(
                np.asarray(inputs[xk]), np.asarray(inputs[eik]),
                np.asarray(inputs[ek]), np.asarray(inputs[uk]), core)
            m[f"xh{s}"] = xh
            m[f"eh{s}"] = ehm
            m[f"dsT{s}"] = dsT
            m[f"de{s}"] = dem
            m[f"uT{s}"] = uT
            b3 = np.asarray(inputs["edge_mlp"][2][1], np.float32)
            g0 = GPC * core
            dstv = np.asarray(inputs[eik][1], np.int64)
            ac = np.zeros((NQ * 128, 128), np.float32)
            for g in range(GPC):
                q, mm_ = g // 4, g % 4
                dg = dstv[EPG * (g0 + g):EPG * (g0 + g + 1)] - NPG * (g0 + g)
                indeg = np.bincount(dg, minlength=NPG).astype(np.float32)
                ac[128 * q:128 * (q + 1), 32 * mm_:32 * (mm_ + 1)] = \
                    np.outer(indeg, b3)
            m[f"aggcorr{s}"] = ac
        in_maps.append(m)
    return in_maps


def _execute(inputs, trace=False):
    nc = _get_nc()
    in_maps = _build_in_maps(inputs)
    res = run_bass_kernel_spmd(nc, in_maps, core_ids=list(range(NCORES)),
                               trace=trace)
    full = np.zeros((NSTEPS, B, F), np.float32)
    for core in range(NCORES):
        o = np.asarray(res.results[core]["out"], np.float32).reshape(NSTEPS, F, GPC)
        for g in range(GPC):
            full[:, GPC * core + g, :] = o[:, :, _SLOT[g]]
    return full, res


def kernel(**inputs):
    out, _ = _execute(inputs, trace=False)
    return out
